# revision 31
# baseline (speedup 1.0000x reference)
"""AdaGAE processor kernel for 8 Trainium2 NeuronCores.

Row-shards the n dimension across 8 cores. Each core computes its
[n/8, n] stripe of the adjacency kernel, plus partial g = adj @ h via
PE block-matmuls, with two small collectives (AllGather of per-row sort
stats, ReduceScatter of partial g, AllReduce of centroid aggregates).

Math notes (vs the reference):
 - a_i = 33rd-smallest distance in row i, b_i = sum of 32 smallest.
   Found via max8/match_replace hierarchical selection on s = 2G - |x_j|^2
   (row-wise ordering of s is exactly the reverse of d^2).
 - denom_i = (a_i+1e-10)*k - b_i + 1e-10;  s1_i = 1/(2*denom_i)
 - sym adagae (uu region) = relu(s1_i*(a_i' - d)) + relu(q_j*(a_j' - d)),
   q_j = u_j * s1_j  -> no transpose needed, only allgathered row vectors.
 - kernel = semantic + fc*(mixed region) + [uu]*adagae. Region masking is
   done with +BIG terms folded into the matmul aug row (known columns) and
   the sqrt bias (known rows), which push d' so high every relu dies.
 - d = sqrt(xsq_i*(1+2e-5)+1e-6 + xsq_j - 2*x_i.x_j): tiny consistent
   inflation guards sqrt(negative) on the diagonal without a clamp pass.
"""

import sys

if "/opt/trn_rl_repo" not in sys.path:
    sys.path.insert(0, "/opt/trn_rl_repo")

import numpy as np
import ml_dtypes

import concourse.bass as bass
import concourse.bacc as bacc
import concourse.mybir as mybir
from concourse import tile
from concourse import bass_utils


# --- workaround: this container's walrus rejects CTRL instructions with >1
# sync wait; split the kernel-tail drain's waits across single-wait drains. ---
from concourse.vector_clock import ScopedClock as _ScopedClock


_WAIT_LIM = 1


def _split_excess_waits(nc_):
    # this walrus build rejects instructions with >_WAIT_LIM sync waits;
    # carry the excess on engine NOPs inserted just before the instruction.
    f = nc_.m.functions[0]

    ws = nc_._ws_sem

    def make_nop(eng):
        nop = nc_.engines[eng].wait_ge(ws, 0)
        mi = nop.ins
        for b2 in f.blocks:
            il2 = b2.instructions
            if il2 and il2[-1].name == mi.name:
                il2.pop()
                b2.instructions = il2
                return mi
        raise RuntimeError("nop not found in any block")

    for bb in f.blocks:
        il = list(bb.instructions)
        out = []
        changed = False
        for inst in il:
            si = getattr(inst, "sync_info", None)
            waits = list(si.on_wait) if (si is not None and si.on_wait) else []
            if len(waits) > _WAIT_LIM:
                changed = True
                extra, keep = waits[:-_WAIT_LIM], waits[-_WAIT_LIM:]
                for i in range(0, len(extra), _WAIT_LIM):
                    mi = make_nop(inst.engine)
                    mi.sync_info = mybir.SyncInfo(
                        on_wait=extra[i:i + _WAIT_LIM], on_update=[])
                    out.append(mi)
                inst.sync_info = mybir.SyncInfo(
                    on_wait=keep, on_update=list(si.on_update or []))
            out.append(inst)
        if changed:
            bb.instructions = out


def _split_drain_and_barrier(self, tick_clock, wait_clock):
    nc_ = self.nc
    drain_inst = nc_.sync.drain()
    wait_clock.add_sem_waits(
        drain_inst.ins, _ScopedClock({None: tick_clock.global_clock}))
    mi = drain_inst.ins
    si = mi.sync_info
    if si is not None and si.on_wait and len(si.on_wait) > 1:
        waits = list(si.on_wait)
        mi.sync_info = mybir.SyncInfo(on_wait=[waits[0]], on_update=list(si.on_update or []))
        for w in waits[1:]:
            d2 = nc_.sync.drain()
            d2.ins.sync_info = mybir.SyncInfo(on_wait=[w], on_update=[])
    _split_excess_waits(nc_)
    nc_.all_engine_barrier()
    assert self.sems is not None
    popped = nc_._tile_sem_poison_stack.pop()
    assert popped is self._sem_poison
    nc_.clear_and_free_semaphores(list(self.sems.allocated().values()))
    nc_.all_engine_barrier()


tile.TileContext._drain_and_barrier = _split_drain_and_barrier

F32 = mybir.dt.float32
F32R = mybir.dt.float32r
BF16 = mybir.dt.bfloat16
AF = mybir.ActivationFunctionType
ALU = mybir.AluOpType

BIG = 1.0e6
INFL = 2.0e-5  # relative inflation of xsq_i in d^2 (diag sqrt guard)
GUARD = 0.25  # absolute d^2 floor: covers fp32r matmul rounding on the diagonal
NEG = -3.0e38


def r32(ap):
    return ap.bitcast(F32R)


def build(N=8192, CORES=8, H=128, NCLS=10, K=32, CC=512, SEL=256):
    ROWS = N // CORES
    RB = ROWS // 128
    NCC = N // CC
    NSEL = CC // SEL
    NCAND = NCC * NSEL * 8
    NR = (K + 8) // 8  # selection rounds (5 for K=32 -> top-40)
    NTOP = NR * 8
    assert NTOP > K
    RG = [list(range(CORES))]

    nc = bass.Bass("TRN2", target_bir_lowering=False, num_devices=CORES)
    _ws_ctx = nc.semaphore("waitsplit")
    nc._ws_sem = _ws_ctx.__enter__()

    # ---------------- I/O ----------------
    xt = nc.dram_tensor("xt", [H, N], F32R, kind="ExternalInput")
    x2t = nc.dram_tensor("x2t", [H, ROWS], F32R, kind="ExternalInput")
    aug1 = nc.dram_tensor("aug1", [1, N], F32R, kind="ExternalInput")
    aug2 = nc.dram_tensor("aug2", [1, N], F32R, kind="ExternalInput")
    rh12 = nc.dram_tensor("rh12", [12, N], BF16, kind="ExternalInput")
    lh12 = nc.dram_tensor("lh12", [12, ROWS], BF16, kind="ExternalInput")
    bias0 = nc.dram_tensor("bias0", [128, RB], F32, kind="ExternalInput")
    bias2 = nc.dram_tensor("bias2", [128, RB], F32, kind="ExternalInput")
    uloc = nc.dram_tensor("uloc", [128, RB], F32, kind="ExternalInput")
    hloc = nc.dram_tensor("hloc", [128, RB * 2], F32R, kind="ExternalInput")
    ohml = nc.dram_tensor("ohml", [128, RB * NCLS], F32R, kind="ExternalInput")
    invc2 = nc.dram_tensor("invc2", [2, NCLS], F32, kind="ExternalInput")
    bgh = nc.dram_tensor("bgh", [2, N], F32, kind="ExternalInput")

    o_adj = nc.dram_tensor("o_adj", [ROWS, N], BF16, kind="ExternalOutput")
    o_g = nc.dram_tensor("o_g", [ROWS, 2], F32, kind="ExternalOutput")
    o_sc = nc.dram_tensor("o_sc", [ROWS, NCLS], F32, kind="ExternalOutput")

    with tile.TileContext(nc) as tc:
        with (
            tc.tile_pool(name="const", bufs=1) as cpool,
            tc.tile_pool(name="dram", bufs=1, space="DRAM") as dpool,
        ):
            # persistent SBUF tiles
            xt_sb = cpool.tile([H, N], F32R)
            x2t_sb = cpool.tile([H, ROWS], F32R)
            lh12_sb = cpool.tile([12, ROWS], BF16)
            bias0_sb = cpool.tile([128, RB], F32)
            bias2_sb = cpool.tile([128, RB], F32)
            uloc_sb = cpool.tile([128, RB], F32)
            hloc_sb = cpool.tile([128, RB * 2], F32R)
            ohml_sb = cpool.tile([128, RB * NCLS], F32R)
            invc2_sb = cpool.tile([2, NCLS], F32)
            ones_f32 = cpool.tile([1, 128], F32)
            ones_sb = cpool.tile([1, 128], F32R)
            # per-row stats (phase1 -> phase2)
            a1b_sb = cpool.tile([128, RB], F32)  # (a+eps)*s1
            ns1_sb = cpool.tile([128, RB], F32)  # -s1
            q_sb = cpool.tile([128, RB], F32R)  # u*s1
            qa_sb = cpool.tile([128, RB], F32R)  # (a+eps)*u*s1

            for t, src in [
                (xt_sb, xt), (x2t_sb, x2t),
                (lh12_sb, lh12), (bias0_sb, bias0),
                (bias2_sb, bias2), (uloc_sb, uloc), (hloc_sb, hloc),
                (ohml_sb, ohml), (invc2_sb, invc2),
            ]:
                nc.sync.dma_start(t[:, :], src[:, :])
            nc.vector.memset(ones_f32[:, :], 1.0)
            nc.vector.tensor_copy(ones_sb[:, :], ones_f32[:, :])

            # DRAM bounce buffers for collectives
            gb_in = dpool.tile([1, 2 * ROWS], F32R)
            gb = dpool.tile([CORES, 2 * ROWS], F32R)
            gpart0 = dpool.tile([1, N], F32)
            gpart1 = dpool.tile([1, N], F32)
            grs0 = dpool.tile([1, ROWS], F32)
            grs1 = dpool.tile([1, ROWS], F32)
            aggd = dpool.tile([NCLS, 2], F32)
            aggf = dpool.tile([NCLS, 2], F32)

            # ================= PHASE 1: row stats =================
            with (
                tc.tile_pool(name="p1psum", bufs=2, space="PSUM") as pp1,
                tc.tile_pool(name="p1sb", bufs=3) as sp1,
                tc.tile_pool(name="p1small", bufs=3) as sm1,
            ):
                for rb in range(RB):
                    cands = sp1.tile([128, NCAND], F32, tag="cands")
                    for cc in range(NCC):
                        ps = pp1.tile([128, CC], F32, tag="ps1")
                        a1row = sp1.tile([1, CC], F32R, tag="a1row")
                        nc.sync.dma_start(a1row[:, :], aug1[:, cc * CC:(cc + 1) * CC])
                        nc.tensor.matmul(
                            ps[:, :], r32(x2t_sb[:, rb * 128:(rb + 1) * 128]),
                            r32(xt_sb[:, cc * CC:(cc + 1) * CC]),
                            start=True, stop=False)
                        nc.tensor.matmul(
                            ps[:, :], r32(ones_sb[:, :]), r32(a1row[:, :]),
                            start=False, stop=True)
                        sch = sp1.tile([128, CC], F32, tag="sch")
                        nc.scalar.copy(sch[:, :], ps[:, :])
                        for sl in range(NSEL):
                            nc.vector.max(
                                cands[:, (cc * NSEL + sl) * 8:(cc * NSEL + sl + 1) * 8],
                                sch[:, sl * SEL:(sl + 1) * SEL])
                    # stage B: top-NTOP of candidates
                    maxima = sm1.tile([128, NTOP], F32, tag="maxima")
                    work = cands
                    for r in range(NR):
                        nc.vector.max(maxima[:, r * 8:(r + 1) * 8], work[:, :])
                        if r < NR - 1:
                            nwork = sp1.tile([128, NCAND], F32, tag="cands")
                            nc.vector.match_replace(
                                nwork[:, :], maxima[:, r * 8:(r + 1) * 8],
                                work[:, :], NEG)
                            work = nwork
                    # stats
                    d2t = sm1.tile([128, NTOP], F32, tag="d2t")
                    nc.vector.tensor_scalar(
                        d2t[:, :], maxima[:, :], -1.0, bias0_sb[:, rb:rb + 1],
                        ALU.mult, ALU.add)
                    d40 = sm1.tile([128, NTOP], F32, tag="d40")
                    nc.scalar.activation(d40[:, :], d2t[:, :], AF.Sqrt)
                    bsum = sm1.tile([128, 1], F32, tag="bsum")
                    nc.vector.reduce_sum(bsum[:, :], d40[:, 0:K], axis=mybir.AxisListType.X)
                    akt = sm1.tile([128, 1], F32, tag="akt")
                    nc.vector.tensor_scalar(
                        akt[:, :], d40[:, K:K + 1], float(K), (K + 1) * 1e-10,
                        ALU.mult, ALU.add)
                    den = sm1.tile([128, 1], F32, tag="den")
                    nc.vector.tensor_sub(den[:, :], akt[:, :], bsum[:, :])
                    dd2 = sm1.tile([128, 1], F32, tag="dd2")
                    nc.vector.tensor_scalar_mul(dd2[:, :], den[:, :], 2.0)
                    s1t = sm1.tile([128, 1], F32, tag="s1t")
                    nc.vector.reciprocal(s1t[:, :], dd2[:, :])
                    aep = sm1.tile([128, 1], F32, tag="aep")
                    nc.vector.tensor_scalar_add(aep[:, :], d40[:, K:K + 1], 1e-10)
                    nc.vector.tensor_mul(a1b_sb[:, rb:rb + 1], aep[:, :], s1t[:, :])
                    nc.vector.tensor_scalar_mul(ns1_sb[:, rb:rb + 1], s1t[:, :], -1.0)
                    nc.vector.tensor_mul(q_sb[:, rb:rb + 1], s1t[:, :], uloc_sb[:, rb:rb + 1])
                    nc.vector.tensor_mul(qa_sb[:, rb:rb + 1], aep[:, :], q_sb[:, rb:rb + 1])

            # ship q/qa to DRAM bounce, allgather
            qv = gb_in[0, 0:ROWS].rearrange("(rb p) -> p rb", p=128)
            qav = gb_in[0, ROWS:2 * ROWS].rearrange("(rb p) -> p rb", p=128)
            nc.sync.dma_start(qv, q_sb[:, :])
            nc.sync.dma_start(qav, qa_sb[:, :])
            nc.gpsimd.collective_compute(
                "AllGather", ALU.bypass, replica_groups=RG,
                ins=[gb_in[:, :].opt()], outs=[gb[:, :].opt()])

            # ================= PHASE 2: adjacency stripe =================
            CC2 = min(int(__import__('os').environ.get('KCC2', '2048')), N)
            NCC2 = N // CC2
            SUB = CC2 // 512
            with (
                tc.tile_pool(name="p2ps_s", bufs=2, space="PSUM") as pps,
                tc.tile_pool(name="p2ps_b", bufs=(1 if CC2 >= 2048 else 2), space="PSUM") as ppb,
                tc.tile_pool(name="p2ps_g", bufs=1, space="PSUM") as ppg,
                tc.tile_pool(name="p2sb", bufs=2) as sp2,
                tc.tile_pool(name="p2row", bufs=2) as rp2,
            ):
                for cc in range(NCC2):
                    qbc = sp2.tile([128, CC2], F32, tag="qbc")
                    qabc = sp2.tile([128, CC2], F32, tag="qabc")
                    pos = 0
                    while pos < CC2:
                        j = cc * CC2 + pos
                        dev, off = j // ROWS, j % ROWS
                        L = min(ROWS - off, CC2 - pos)
                        nc.sync.dma_start(
                            qbc[:, pos:pos + L],
                            gb[dev, off:off + L].bitcast(F32).partition_broadcast(128))
                        nc.sync.dma_start(
                            qabc[:, pos:pos + L],
                            gb[dev, ROWS + off:ROWS + off + L].bitcast(F32).partition_broadcast(128))
                        pos += L
                    a2row = rp2.tile([1, CC2], F32R, tag="a2row")
                    r12t = rp2.tile([12, CC2], BF16, tag="r12t")
                    nc.sync.dma_start(a2row[:, :], aug2[:, cc * CC2:(cc + 1) * CC2])
                    nc.sync.dma_start(r12t[:, :], rh12[:, cc * CC2:(cc + 1) * CC2])
                    gpss = [ppg.tile([2, 512], F32, tag=f"gps{s}", name=f"gps{s}_{cc}")
                             for s in range(SUB)]
                    for rb in range(RB):
                        dp = sp2.tile([128, CC2], F32, tag="dp", bufs=3)
                        for sb in range(SUB):
                            ps = pps.tile([128, 512], F32, tag="ps2", bufs=3)
                            c0 = cc * CC2 + sb * 512
                            nc.tensor.matmul(
                                ps[:, :], r32(x2t_sb[:, rb * 128:(rb + 1) * 128]),
                                r32(xt_sb[:, c0:c0 + 512]),
                                start=True, stop=False)
                            nc.tensor.matmul(
                                ps[:, :], r32(ones_sb[:, :]),
                                r32(a2row[:, sb * 512:(sb + 1) * 512]),
                                start=False, stop=True)
                            nc.scalar.activation(
                                dp[:, sb * 512:(sb + 1) * 512], ps[:, :], AF.Sqrt,
                                bias=bias2_sb[:, rb:rb + 1], scale=-1.0)
                        a1 = sp2.tile([128, CC2], F32, tag="a1", bufs=3)
                        nc.scalar.activation(a1[:, :], dp[:, :], AF.Relu,
                                             bias=a1b_sb[:, rb:rb + 1],
                                             scale=ns1_sb[:, rb:rb + 1])
                        mt = sp2.tile([128, CC2], F32R, tag="mt", bufs=3)
                        nc.gpsimd.tensor_tensor(mt[:, :], dp[:, :], qbc[:, :],
                                                op=ALU.mult)
                        nc.vector.scalar_tensor_tensor(
                            mt[:, :], mt[:, :], 0.0, qabc[:, :], ALU.add, ALU.subtract)
                        nc.vector.scalar_tensor_tensor(
                            mt[:, :], mt[:, :], 0.0, a1[:, :], ALU.min, ALU.subtract)
                        adjt = sp2.tile([128, CC2], BF16, tag="adjt", bufs=3)
                        for sb in range(SUB):
                            bps = ppb.tile([128, 512], F32, tag="bps")
                            nc.tensor.matmul(
                                bps[:, :], lh12_sb[:, rb * 128:(rb + 1) * 128],
                                r12t[:, sb * 512:(sb + 1) * 512],
                                start=True, stop=True)
                            nc.vector.tensor_sub(
                                adjt[:, sb * 512:(sb + 1) * 512], bps[:, :],
                                mt[:, sb * 512:(sb + 1) * 512])
                            nc.tensor.matmul(
                                gpss[sb][:, :],
                                r32(hloc_sb[:, rb * 2:(rb + 1) * 2]),
                                mt[:, sb * 512:(sb + 1) * 512],
                                start=(rb == 0), stop=(rb == RB - 1))
                        nc.sync.dma_start(
                            o_adj[rb * 128:(rb + 1) * 128, cc * CC2:(cc + 1) * CC2],
                            adjt[:, :])
                    bght = rp2.tile([2, CC2], F32, tag="bght", bufs=1)
                    nc.sync.dma_start(bght[:, :], bgh[:, cc * CC2:(cc + 1) * CC2])
                    gstage = rp2.tile([2, CC2], F32, tag="gstage", bufs=1)
                    for sb in range(SUB):
                        nc.vector.tensor_sub(gstage[:, sb * 512:(sb + 1) * 512],
                                             bght[:, sb * 512:(sb + 1) * 512],
                                             gpss[sb][:, :])
                    nc.sync.dma_start(gpart0[0, cc * CC2:(cc + 1) * CC2], gstage[0:1, :])
                    nc.sync.dma_start(gpart1[0, cc * CC2:(cc + 1) * CC2], gstage[1:2, :])

            # reduce-scatter partial g to local rows (per component)
            nc.gpsimd.collective_compute(
                "ReduceScatter", ALU.add, replica_groups=RG,
                ins=[gpart0[:, :].opt()], outs=[grs0[:, :].opt()])
            nc.gpsimd.collective_compute(
                "ReduceScatter", ALU.add, replica_groups=RG,
                ins=[gpart1[:, :].opt()], outs=[grs1[:, :].opt()])

            # ================= EPILOGUE =================
            with (
                tc.tile_pool(name="e_ps", bufs=1, space="PSUM") as pe,
                tc.tile_pool(name="e_sb", bufs=1) as se,
            ):
                gpc = se.tile([128, RB * 2], F32)
                nc.sync.dma_start(
                    gpc[:, :].rearrange("p (rb c) -> c p rb", c=2)[0],
                    grs0[0, :].rearrange("(rb p) -> p rb", p=128))
                nc.sync.dma_start(
                    gpc[:, :].rearrange("p (rb c) -> c p rb", c=2)[1],
                    grs1[0, :].rearrange("(rb p) -> p rb", p=128))
                gpct = se.tile([128, RB * 2], F32R)
                nc.scalar.activation(gpct[:, :], gpc[:, :], AF.Tanh)
                nc.sync.dma_start(
                    o_g[:, :].rearrange("(rb p) c -> p rb c", p=128),
                    gpct[:, :].bitcast(F32))
                # centroid partial agg = ohm_loc.T @ tanh(g_loc)
                aggps = pe.tile([NCLS, 2], F32)
                for rb in range(RB):
                    nc.tensor.matmul(
                        aggps[:, :], r32(ohml_sb[:, rb * NCLS:(rb + 1) * NCLS]),
                        r32(gpct[:, rb * 2:(rb + 1) * 2]),
                        start=(rb == 0), stop=(rb == RB - 1))
                aggsb = se.tile([NCLS, 2], F32)
                nc.vector.tensor_copy(aggsb[:, :], aggps[:, :])
                nc.sync.dma_start(aggd[:, :], aggsb[:, :])
                nc.gpsimd.collective_compute(
                    "AllReduce", ALU.add, replica_groups=RG,
                    ins=[aggd[:, :].opt()], outs=[aggf[:, :].opt()])
                aggt = se.tile([2, NCLS], F32)
                nc.sync.dma_start(aggt[:, :], aggf[:, :].rearrange("m c -> c m"))
                ctrt = se.tile([2, NCLS], F32R)
                nc.vector.tensor_mul(ctrt[:, :], aggt[:, :], invc2_sb[:, :])
                sq2 = se.tile([2, NCLS], F32R)
                nc.vector.tensor_mul(sq2[:, :], ctrt[:, :], ctrt[:, :])
                ones2f = se.tile([2, 128], F32)
                ones2 = se.tile([2, 128], F32R)
                nc.vector.memset(ones2f[:, :], 1.0)
                nc.vector.tensor_copy(ones2[:, :], ones2f[:, :])
                crow2 = se.tile([1, 2 * NCLS], F32R)
                nc.sync.dma_start(crow2[0:1, 0:NCLS], ctrt[0:1, :])
                nc.sync.dma_start(crow2[0:1, NCLS:2 * NCLS], ctrt[1:2, :])
                bc = pe.tile([128, 3 * NCLS], F32)
                nc.tensor.matmul(bc[:, 0:NCLS], r32(ones_sb[:, :]), r32(crow2[0:1, 0:NCLS]),
                                 start=True, stop=True, skip_group_check=True)
                nc.tensor.matmul(bc[:, NCLS:2 * NCLS], r32(ones_sb[:, :]),
                                 r32(crow2[0:1, NCLS:2 * NCLS]),
                                 start=True, stop=True, skip_group_check=True)
                nc.tensor.matmul(bc[:, 2 * NCLS:3 * NCLS], r32(ones2[:, :]), r32(sq2[:, :]),
                                 start=True, stop=True, skip_group_check=True)
                c0b, c1b, csb = (bc[:, 0:NCLS], bc[:, NCLS:2 * NCLS],
                                 bc[:, 2 * NCLS:3 * NCLS])
                scall = se.tile([128, RB * NCLS], F32)
                for rb in range(RB):
                    g0 = gpct[:, rb * 2:rb * 2 + 1].bitcast(F32)
                    g1 = gpct[:, rb * 2 + 1:rb * 2 + 2].bitcast(F32)
                    sqg = se.tile([128, 2], F32, tag="sqg")
                    nc.vector.tensor_mul(sqg[:, :], gpct[:, rb * 2:(rb + 1) * 2],
                                         gpct[:, rb * 2:(rb + 1) * 2])
                    gsq = se.tile([128, 1], F32, tag="gsq")
                    nc.vector.reduce_sum(gsq[:, :], sqg[:, :], axis=mybir.AxisListType.X)
                    v1 = se.tile([128, NCLS], F32, tag="v1")
                    nc.vector.tensor_scalar(v1[:, :], c1b, g1, None, ALU.mult)
                    v2 = se.tile([128, NCLS], F32, tag="v2")
                    nc.vector.scalar_tensor_tensor(
                        v2[:, :], c0b, g0, v1[:, :], ALU.mult, ALU.add)
                    v3 = se.tile([128, NCLS], F32, tag="v3")
                    nc.vector.scalar_tensor_tensor(
                        v3[:, :], v2[:, :], -2.0, csb, ALU.mult, ALU.add)
                    v4 = se.tile([128, NCLS], F32, tag="v4")
                    nc.vector.tensor_scalar(
                        v4[:, :], v3[:, :], gsq[:, :], 0.0, ALU.add, ALU.max)
                    v5 = se.tile([128, NCLS], F32, tag="v5")
                    nc.scalar.activation(v5[:, :], v4[:, :], AF.Sqrt)
                    nc.vector.tensor_scalar_mul(
                        scall[:, rb * NCLS:(rb + 1) * NCLS], v5[:, :], -1.0)
                nc.sync.dma_start(
                    o_sc[:, :].rearrange("(rb p) c -> p rb c", p=128), scall[:, :])
    return nc


_CACHE = {}


def _get_nc(N, CORES, H, NCLS, K, CC, SEL):
    key = (N, CORES, H, NCLS, K, CC, SEL)
    if key not in _CACHE:
        _CACHE[key] = build(N, CORES, H, NCLS, K, CC, SEL)
    return _CACHE[key]


def round_fp32r(a):
    b = np.ascontiguousarray(a, np.float32).view(np.uint32)
    lsb = (b >> 12) & 1
    r = (b + 0x7FF + lsb) & 0xFFFFF000
    return r.view(np.float32)


def prep_inputs(x, ohm_labels, W, b, spars, CORES=8):
    N, H = x.shape
    NCLS = ohm_labels.shape[1]
    ROWS = N // CORES
    RB = ROWS // 128
    x = np.asarray(x, np.float32)
    ohm = np.asarray(ohm_labels, np.float32)
    W = np.asarray(W, np.float32)
    b = np.asarray(b, np.float32)
    u = (ohm.sum(1) == 0).astype(np.float32)
    xsq = (x * x).sum(1).astype(np.float32)
    h = (x @ W.T + b).astype(np.float32)
    fc = np.float32(1.0 / max(float(u.sum()), 1.0))
    xt = np.ascontiguousarray(x.T)
    aug1 = (-xsq)[None, :].astype(np.float32)
    aug2 = (-xsq - BIG * (1.0 - u))[None, :].astype(np.float32)
    rh12 = np.concatenate(
        [ohm.T, (fc * (1.0 - u))[None, :], (fc * u)[None, :]], 0).astype(np.float32)
    counts = ohm.sum(0)
    inv = np.where(counts > 0, 1.0 / np.maximum(counts, 1.0), 0.0).astype(np.float32)
    invc2 = np.ascontiguousarray(np.stack([inv, inv], 0))

    def perm(v):  # [ROWS,...] -> [128, RB, ...] -> [128, RB*...]
        r = v.reshape(RB, 128, -1).transpose(1, 0, 2).reshape(128, -1)
        return np.ascontiguousarray(r.astype(np.float32))

    in_maps = []
    for c in range(CORES):
        rs = slice(c * ROWS, (c + 1) * ROWS)
        bias0 = xsq[rs] * (1.0 + INFL) + 1e-6 + GUARD
        hl = h[rs]
        aggl = ohm[rs].T @ hl                     # [NCLS, 2]
        t1 = (1.0 - u[rs]) @ hl                   # [2]
        t2 = u[rs] @ hl                           # [2]
        bghv = (ohm @ aggl + fc * (np.outer(u, t1) + np.outer(1.0 - u, t2)))
        in_maps.append({
            "bgh": np.ascontiguousarray(bghv.T.astype(np.float32)),
            "xt": round_fp32r(xt), "aug1": round_fp32r(aug1),
            "aug2": round_fp32r(aug2), "rh12": rh12.astype(ml_dtypes.bfloat16),
            "invc2": invc2,
            "x2t": round_fp32r(2.0 * x[rs].T),
            "lh12": np.concatenate(
                [ohm[rs].T, u[rs][None, :], (1.0 - u[rs])[None, :]],
                0).astype(ml_dtypes.bfloat16),
            "bias0": perm(bias0),
            "bias2": perm(bias0 + BIG * (1.0 - u[rs])),
            "uloc": perm(u[rs]),
            "hloc": round_fp32r(perm(h[rs])),
            "ohml": round_fp32r(perm(ohm[rs])),
        })
    return in_maps


def kernel(x, ohm_labels, W, b, spars):
    N, H = np.asarray(x).shape
    NCLS = np.asarray(ohm_labels).shape[1]
    CORES = 8
    K = int(spars)
    nc = _get_nc(N, CORES, H, NCLS, K, 512, 512)
    in_maps = prep_inputs(x, ohm_labels, W, b, spars, CORES)
    res = bass_utils.run_bass_kernel_spmd(nc, in_maps, core_ids=list(range(CORES)))
    outs = res.results
    adj = np.concatenate([o["o_adj"] for o in outs], 0).astype(np.float32)
    g = np.concatenate([o["o_g"] for o in outs], 0)
    sc = np.concatenate([o["o_sc"] for o in outs], 0)
    return sc, g, adj


# revision 32
# speedup vs baseline: 1.2199x; 1.2199x over previous
"""AdaGAE processor kernel for 8 Trainium2 NeuronCores.

Row-shards the n dimension across 8 cores. Each core computes its
[n/8, n] stripe of the adjacency kernel, plus partial g = adj @ h via
PE block-matmuls, with two small collectives (AllGather of per-row sort
stats, ReduceScatter of partial g, AllReduce of centroid aggregates).

Math notes (vs the reference):
 - a_i = 33rd-smallest distance in row i, b_i = sum of 32 smallest.
   Found via max8/match_replace hierarchical selection on s = 2G - |x_j|^2
   (row-wise ordering of s is exactly the reverse of d^2).
 - denom_i = (a_i+1e-10)*k - b_i + 1e-10;  s1_i = 1/(2*denom_i)
 - sym adagae (uu region) = relu(s1_i*(a_i' - d)) + relu(q_j*(a_j' - d)),
   q_j = u_j * s1_j  -> no transpose needed, only allgathered row vectors.
 - kernel = semantic + fc*(mixed region) + [uu]*adagae. Region masking is
   done with +BIG terms folded into the matmul aug row (known columns) and
   the sqrt bias (known rows), which push d' so high every relu dies.
 - d = sqrt(xsq_i*(1+2e-5)+1e-6 + xsq_j - 2*x_i.x_j): tiny consistent
   inflation guards sqrt(negative) on the diagonal without a clamp pass.
"""

import sys

if "/opt/trn_rl_repo" not in sys.path:
    sys.path.insert(0, "/opt/trn_rl_repo")

import numpy as np
import ml_dtypes

import concourse.bass as bass
import concourse.bacc as bacc
import concourse.mybir as mybir
from concourse import tile
from concourse import bass_utils


# --- workaround: this container's walrus rejects CTRL instructions with >1
# sync wait; split the kernel-tail drain's waits across single-wait drains. ---
from concourse.vector_clock import ScopedClock as _ScopedClock


_WAIT_LIM = 1


def _split_excess_waits(nc_):
    # this walrus build rejects instructions with >_WAIT_LIM sync waits;
    # carry the excess on engine NOPs inserted just before the instruction.
    f = nc_.m.functions[0]

    ws = nc_._ws_sem

    def make_nop(eng):
        nop = nc_.engines[eng].wait_ge(ws, 0)
        mi = nop.ins
        for b2 in f.blocks:
            il2 = b2.instructions
            if il2 and il2[-1].name == mi.name:
                il2.pop()
                b2.instructions = il2
                return mi
        raise RuntimeError("nop not found in any block")

    for bb in f.blocks:
        il = list(bb.instructions)
        out = []
        changed = False
        for inst in il:
            si = getattr(inst, "sync_info", None)
            waits = list(si.on_wait) if (si is not None and si.on_wait) else []
            if len(waits) > _WAIT_LIM:
                changed = True
                extra, keep = waits[:-_WAIT_LIM], waits[-_WAIT_LIM:]
                for i in range(0, len(extra), _WAIT_LIM):
                    mi = make_nop(inst.engine)
                    mi.sync_info = mybir.SyncInfo(
                        on_wait=extra[i:i + _WAIT_LIM], on_update=[])
                    out.append(mi)
                inst.sync_info = mybir.SyncInfo(
                    on_wait=keep, on_update=list(si.on_update or []))
            out.append(inst)
        if changed:
            bb.instructions = out


def _split_drain_and_barrier(self, tick_clock, wait_clock):
    nc_ = self.nc
    drain_inst = nc_.sync.drain()
    wait_clock.add_sem_waits(
        drain_inst.ins, _ScopedClock({None: tick_clock.global_clock}))
    mi = drain_inst.ins
    si = mi.sync_info
    if si is not None and si.on_wait and len(si.on_wait) > 1:
        waits = list(si.on_wait)
        mi.sync_info = mybir.SyncInfo(on_wait=[waits[0]], on_update=list(si.on_update or []))
        for w in waits[1:]:
            d2 = nc_.sync.drain()
            d2.ins.sync_info = mybir.SyncInfo(on_wait=[w], on_update=[])
    _split_excess_waits(nc_)
    nc_.all_engine_barrier()
    assert self.sems is not None
    popped = nc_._tile_sem_poison_stack.pop()
    assert popped is self._sem_poison
    nc_.clear_and_free_semaphores(list(self.sems.allocated().values()))
    nc_.all_engine_barrier()


tile.TileContext._drain_and_barrier = _split_drain_and_barrier

F32 = mybir.dt.float32
F32R = mybir.dt.float32r
BF16 = mybir.dt.bfloat16
AF = mybir.ActivationFunctionType
ALU = mybir.AluOpType

BIG = 1.0e6
INFL = 2.0e-5  # relative inflation of xsq_i in d^2 (diag sqrt guard)
GUARD = 0.25  # absolute d^2 floor: covers fp32r matmul rounding on the diagonal
NEG = -3.0e38


def r32(ap):
    return ap.bitcast(F32R)


def build(N=8192, CORES=8, H=128, NCLS=10, K=32, CC=512, SEL=256):
    ROWS = N // CORES
    RB = ROWS // 128
    NCC = N // CC
    NSEL = CC // SEL
    NCAND = NCC * NSEL * 8
    NR = (K + 8) // 8  # selection rounds (5 for K=32 -> top-40)
    NTOP = NR * 8
    assert NTOP > K
    RG = [list(range(CORES))]

    nc = bass.Bass("TRN2", target_bir_lowering=False, num_devices=CORES)
    _ws_ctx = nc.semaphore("waitsplit")
    nc._ws_sem = _ws_ctx.__enter__()

    # ---------------- I/O ----------------
    xt = nc.dram_tensor("xt", [H, N], F32R, kind="ExternalInput")
    x2t = nc.dram_tensor("x2t", [H, ROWS], F32R, kind="ExternalInput")
    aug1 = nc.dram_tensor("aug1", [1, N], F32R, kind="ExternalInput")
    aug2 = nc.dram_tensor("aug2", [1, N], F32R, kind="ExternalInput")
    rh12 = nc.dram_tensor("rh12", [12, N], BF16, kind="ExternalInput")
    lh12 = nc.dram_tensor("lh12", [12, ROWS], BF16, kind="ExternalInput")
    bias0 = nc.dram_tensor("bias0", [128, RB], F32, kind="ExternalInput")
    bias2 = nc.dram_tensor("bias2", [128, RB], F32, kind="ExternalInput")
    uloc = nc.dram_tensor("uloc", [128, RB], F32, kind="ExternalInput")
    hloc = nc.dram_tensor("hloc", [128, RB * 2], F32R, kind="ExternalInput")
    ohml = nc.dram_tensor("ohml", [128, RB * NCLS], F32R, kind="ExternalInput")
    invc2 = nc.dram_tensor("invc2", [2, NCLS], F32, kind="ExternalInput")
    bgh = nc.dram_tensor("bgh", [2, N], F32, kind="ExternalInput")

    o_adj = nc.dram_tensor("o_adj", [ROWS, N], BF16, kind="ExternalOutput")
    o_g = nc.dram_tensor("o_g", [ROWS, 2], F32, kind="ExternalOutput")
    o_sc = nc.dram_tensor("o_sc", [ROWS, NCLS], F32, kind="ExternalOutput")

    with tile.TileContext(nc) as tc:
        with (
            tc.tile_pool(name="const", bufs=1) as cpool,
            tc.tile_pool(name="dram", bufs=1, space="DRAM") as dpool,
        ):
            # persistent SBUF tiles
            xt_sb = cpool.tile([H, N], F32R)
            x2t_sb = cpool.tile([H, ROWS], F32R)
            lh12_sb = cpool.tile([12, ROWS], BF16)
            bias0_sb = cpool.tile([128, RB], F32)
            bias2_sb = cpool.tile([128, RB], F32)
            uloc_sb = cpool.tile([128, RB], F32)
            hloc_sb = cpool.tile([128, RB * 2], F32R)
            ohml_sb = cpool.tile([128, RB * NCLS], F32R)
            invc2_sb = cpool.tile([2, NCLS], F32)
            ones_f32 = cpool.tile([1, 128], F32)
            ones_sb = cpool.tile([1, 128], F32R)
            # per-row stats (phase1 -> phase2)
            a1b_sb = cpool.tile([128, RB], F32)  # (a+eps)*s1
            ns1_sb = cpool.tile([128, RB], F32)  # -s1
            q_sb = cpool.tile([128, RB], F32R)  # u*s1
            qa_sb = cpool.tile([128, RB], F32R)  # (a+eps)*u*s1

            for t, src in [
                (xt_sb, xt), (x2t_sb, x2t),
                (lh12_sb, lh12), (bias0_sb, bias0),
                (bias2_sb, bias2), (uloc_sb, uloc), (hloc_sb, hloc),
                (ohml_sb, ohml), (invc2_sb, invc2),
            ]:
                nc.sync.dma_start(t[:, :], src[:, :])
            nc.vector.memset(ones_f32[:, :], 1.0)
            nc.vector.tensor_copy(ones_sb[:, :], ones_f32[:, :])

            # DRAM bounce buffers for collectives
            gb_in = dpool.tile([1, 2 * ROWS], F32R)
            gb = dpool.tile([CORES, 2 * ROWS], F32R)
            gpart0 = dpool.tile([1, N], F32)
            gpart1 = dpool.tile([1, N], F32)
            grs0 = dpool.tile([1, ROWS], F32)
            grs1 = dpool.tile([1, ROWS], F32)
            aggd = dpool.tile([NCLS, 2], F32)
            aggf = dpool.tile([NCLS, 2], F32)

            # ================= PHASE 1: row stats =================
            with (
                tc.tile_pool(name="p1psum", bufs=2, space="PSUM") as pp1,
                tc.tile_pool(name="p1sb", bufs=3) as sp1,
                tc.tile_pool(name="p1small", bufs=3) as sm1,
            ):
                for rb in range(RB):
                    cands = sp1.tile([128, NCAND], F32, tag="cands")
                    for cc in range(NCC):
                        ps = pp1.tile([128, CC], F32, tag="ps1")
                        a1row = sp1.tile([1, CC], F32R, tag="a1row")
                        nc.sync.dma_start(a1row[:, :], aug1[:, cc * CC:(cc + 1) * CC])
                        nc.tensor.matmul(
                            ps[:, :], r32(x2t_sb[:, rb * 128:(rb + 1) * 128]),
                            r32(xt_sb[:, cc * CC:(cc + 1) * CC]),
                            start=True, stop=False)
                        nc.tensor.matmul(
                            ps[:, :], r32(ones_sb[:, :]), r32(a1row[:, :]),
                            start=False, stop=True)
                        sch = sp1.tile([128, CC], F32, tag="sch")
                        nc.scalar.copy(sch[:, :], ps[:, :])
                        for sl in range(NSEL):
                            nc.vector.max(
                                cands[:, (cc * NSEL + sl) * 8:(cc * NSEL + sl + 1) * 8],
                                sch[:, sl * SEL:(sl + 1) * SEL])
                    # stage B: top-NTOP of candidates
                    maxima = sm1.tile([128, NTOP], F32, tag="maxima")
                    work = cands
                    for r in range(NR):
                        nc.vector.max(maxima[:, r * 8:(r + 1) * 8], work[:, :])
                        if r < NR - 1:
                            nwork = sp1.tile([128, NCAND], F32, tag="cands")
                            nc.vector.match_replace(
                                nwork[:, :], maxima[:, r * 8:(r + 1) * 8],
                                work[:, :], NEG)
                            work = nwork
                    # stats
                    d2t = sm1.tile([128, NTOP], F32, tag="d2t")
                    nc.vector.tensor_scalar(
                        d2t[:, :], maxima[:, :], -1.0, bias0_sb[:, rb:rb + 1],
                        ALU.mult, ALU.add)
                    d40 = sm1.tile([128, NTOP], F32, tag="d40")
                    nc.scalar.activation(d40[:, :], d2t[:, :], AF.Sqrt)
                    bsum = sm1.tile([128, 1], F32, tag="bsum")
                    nc.vector.reduce_sum(bsum[:, :], d40[:, 0:K], axis=mybir.AxisListType.X)
                    akt = sm1.tile([128, 1], F32, tag="akt")
                    nc.vector.tensor_scalar(
                        akt[:, :], d40[:, K:K + 1], float(K), (K + 1) * 1e-10,
                        ALU.mult, ALU.add)
                    den = sm1.tile([128, 1], F32, tag="den")
                    nc.vector.tensor_sub(den[:, :], akt[:, :], bsum[:, :])
                    dd2 = sm1.tile([128, 1], F32, tag="dd2")
                    nc.vector.tensor_scalar_mul(dd2[:, :], den[:, :], 2.0)
                    s1t = sm1.tile([128, 1], F32, tag="s1t")
                    nc.vector.reciprocal(s1t[:, :], dd2[:, :])
                    aep = sm1.tile([128, 1], F32, tag="aep")
                    nc.vector.tensor_scalar_add(aep[:, :], d40[:, K:K + 1], 1e-10)
                    nc.vector.tensor_mul(a1b_sb[:, rb:rb + 1], aep[:, :], s1t[:, :])
                    nc.vector.tensor_scalar_mul(ns1_sb[:, rb:rb + 1], s1t[:, :], -1.0)
                    nc.vector.tensor_mul(q_sb[:, rb:rb + 1], s1t[:, :], uloc_sb[:, rb:rb + 1])
                    nc.vector.tensor_mul(qa_sb[:, rb:rb + 1], aep[:, :], q_sb[:, rb:rb + 1])

            # ship q/qa to DRAM bounce, allgather
            qv = gb_in[0, 0:ROWS].rearrange("(rb p) -> p rb", p=128)
            qav = gb_in[0, ROWS:2 * ROWS].rearrange("(rb p) -> p rb", p=128)
            nc.sync.dma_start(qv, q_sb[:, :])
            nc.sync.dma_start(qav, qa_sb[:, :])
            nc.gpsimd.collective_compute(
                "AllGather", ALU.bypass, replica_groups=RG,
                ins=[gb_in[:, :].opt()], outs=[gb[:, :].opt()])

            # ================= PHASE 2: adjacency stripe =================
            CC2 = min(int(__import__('os').environ.get('KCC2', '2048')), N)
            NCC2 = N // CC2
            SUB = CC2 // 512
            with (
                tc.tile_pool(name="p2ps_s", bufs=2, space="PSUM") as pps,
                tc.tile_pool(name="p2ps_b", bufs=(1 if CC2 >= 2048 else 2), space="PSUM") as ppb,
                tc.tile_pool(name="p2ps_g", bufs=1, space="PSUM") as ppg,
                tc.tile_pool(name="p2sb", bufs=2) as sp2,
                tc.tile_pool(name="p2row", bufs=2) as rp2,
            ):
                for cc in range(NCC2):
                    qbc = sp2.tile([128, CC2], F32, tag="qbc")
                    qabc = sp2.tile([128, CC2], F32, tag="qabc")
                    pos = 0
                    while pos < CC2:
                        j = cc * CC2 + pos
                        dev, off = j // ROWS, j % ROWS
                        L = min(ROWS - off, CC2 - pos)
                        nc.sync.dma_start(
                            qbc[:, pos:pos + L],
                            gb[dev, off:off + L].bitcast(F32).partition_broadcast(128))
                        nc.sync.dma_start(
                            qabc[:, pos:pos + L],
                            gb[dev, ROWS + off:ROWS + off + L].bitcast(F32).partition_broadcast(128))
                        pos += L
                    a2row = rp2.tile([1, CC2], F32R, tag="a2row")
                    r12t = rp2.tile([12, CC2], BF16, tag="r12t")
                    nc.sync.dma_start(a2row[:, :], aug2[:, cc * CC2:(cc + 1) * CC2])
                    nc.sync.dma_start(r12t[:, :], rh12[:, cc * CC2:(cc + 1) * CC2])
                    gpss = [ppg.tile([2, 512], F32, tag=f"gps{s}", name=f"gps{s}_{cc}")
                             for s in range(SUB)]
                    for rb in range(RB):
                        dp = sp2.tile([128, CC2], F32, tag="dp")
                        for sb in range(SUB):
                            ps = pps.tile([128, 512], F32, tag="ps2")
                            c0 = cc * CC2 + sb * 512
                            nc.tensor.matmul(
                                ps[:, :], r32(x2t_sb[:, rb * 128:(rb + 1) * 128]),
                                r32(xt_sb[:, c0:c0 + 512]),
                                start=True, stop=False)
                            nc.tensor.matmul(
                                ps[:, :], r32(ones_sb[:, :]),
                                r32(a2row[:, sb * 512:(sb + 1) * 512]),
                                start=False, stop=True)
                            nc.scalar.activation(
                                dp[:, sb * 512:(sb + 1) * 512], ps[:, :], AF.Sqrt,
                                bias=bias2_sb[:, rb:rb + 1], scale=-1.0)
                        a1 = sp2.tile([128, CC2], F32, tag="a1")
                        nc.scalar.activation(a1[:, :], dp[:, :], AF.Relu,
                                             bias=a1b_sb[:, rb:rb + 1],
                                             scale=ns1_sb[:, rb:rb + 1])
                        mt = sp2.tile([128, CC2], F32R, tag="mt")
                        nc.gpsimd.tensor_tensor(mt[:, :], dp[:, :], qbc[:, :],
                                                op=ALU.mult)
                        nc.vector.scalar_tensor_tensor(
                            mt[:, :], mt[:, :], 0.0, qabc[:, :], ALU.add, ALU.subtract)
                        nc.vector.scalar_tensor_tensor(
                            mt[:, :], mt[:, :], 0.0, a1[:, :], ALU.min, ALU.subtract)
                        adjt = sp2.tile([128, CC2], BF16, tag="adjt")
                        for sb in range(SUB):
                            bps = ppb.tile([128, 512], F32, tag="bps")
                            nc.tensor.matmul(
                                bps[:, :], lh12_sb[:, rb * 128:(rb + 1) * 128],
                                r12t[:, sb * 512:(sb + 1) * 512],
                                start=True, stop=True)
                            nc.vector.tensor_sub(
                                adjt[:, sb * 512:(sb + 1) * 512], bps[:, :],
                                mt[:, sb * 512:(sb + 1) * 512])
                            nc.tensor.matmul(
                                gpss[sb][:, :],
                                r32(hloc_sb[:, rb * 2:(rb + 1) * 2]),
                                mt[:, sb * 512:(sb + 1) * 512],
                                start=(rb == 0), stop=(rb == RB - 1))
                        nc.sync.dma_start(
                            o_adj[rb * 128:(rb + 1) * 128, cc * CC2:(cc + 1) * CC2],
                            adjt[:, :])
                    bght = rp2.tile([2, CC2], F32, tag="bght", bufs=1)
                    nc.sync.dma_start(bght[:, :], bgh[:, cc * CC2:(cc + 1) * CC2])
                    gstage = rp2.tile([2, CC2], F32, tag="gstage", bufs=1)
                    for sb in range(SUB):
                        nc.vector.tensor_sub(gstage[:, sb * 512:(sb + 1) * 512],
                                             bght[:, sb * 512:(sb + 1) * 512],
                                             gpss[sb][:, :])
                    nc.sync.dma_start(gpart0[0, cc * CC2:(cc + 1) * CC2], gstage[0:1, :])
                    nc.sync.dma_start(gpart1[0, cc * CC2:(cc + 1) * CC2], gstage[1:2, :])

            # reduce-scatter partial g to local rows (per component)
            nc.gpsimd.collective_compute(
                "ReduceScatter", ALU.add, replica_groups=RG,
                ins=[gpart0[:, :].opt()], outs=[grs0[:, :].opt()])
            nc.gpsimd.collective_compute(
                "ReduceScatter", ALU.add, replica_groups=RG,
                ins=[gpart1[:, :].opt()], outs=[grs1[:, :].opt()])

            # ================= EPILOGUE =================
            with (
                tc.tile_pool(name="e_ps", bufs=1, space="PSUM") as pe,
                tc.tile_pool(name="e_sb", bufs=1) as se,
            ):
                gpc = se.tile([128, RB * 2], F32)
                nc.sync.dma_start(
                    gpc[:, :].rearrange("p (rb c) -> c p rb", c=2)[0],
                    grs0[0, :].rearrange("(rb p) -> p rb", p=128))
                nc.sync.dma_start(
                    gpc[:, :].rearrange("p (rb c) -> c p rb", c=2)[1],
                    grs1[0, :].rearrange("(rb p) -> p rb", p=128))
                gpct = se.tile([128, RB * 2], F32R)
                nc.scalar.activation(gpct[:, :], gpc[:, :], AF.Tanh)
                nc.sync.dma_start(
                    o_g[:, :].rearrange("(rb p) c -> p rb c", p=128),
                    gpct[:, :].bitcast(F32))
                # centroid partial agg = ohm_loc.T @ tanh(g_loc)
                aggps = pe.tile([NCLS, 2], F32)
                for rb in range(RB):
                    nc.tensor.matmul(
                        aggps[:, :], r32(ohml_sb[:, rb * NCLS:(rb + 1) * NCLS]),
                        r32(gpct[:, rb * 2:(rb + 1) * 2]),
                        start=(rb == 0), stop=(rb == RB - 1))
                aggsb = se.tile([NCLS, 2], F32)
                nc.vector.tensor_copy(aggsb[:, :], aggps[:, :])
                nc.sync.dma_start(aggd[:, :], aggsb[:, :])
                nc.gpsimd.collective_compute(
                    "AllReduce", ALU.add, replica_groups=RG,
                    ins=[aggd[:, :].opt()], outs=[aggf[:, :].opt()])
                aggt = se.tile([2, NCLS], F32)
                nc.sync.dma_start(aggt[:, :], aggf[:, :].rearrange("m c -> c m"))
                ctrt = se.tile([2, NCLS], F32R)
                nc.vector.tensor_mul(ctrt[:, :], aggt[:, :], invc2_sb[:, :])
                sq2 = se.tile([2, NCLS], F32R)
                nc.vector.tensor_mul(sq2[:, :], ctrt[:, :], ctrt[:, :])
                ones2f = se.tile([2, 128], F32)
                ones2 = se.tile([2, 128], F32R)
                nc.vector.memset(ones2f[:, :], 1.0)
                nc.vector.tensor_copy(ones2[:, :], ones2f[:, :])
                crow2 = se.tile([1, 2 * NCLS], F32R)
                nc.sync.dma_start(crow2[0:1, 0:NCLS], ctrt[0:1, :])
                nc.sync.dma_start(crow2[0:1, NCLS:2 * NCLS], ctrt[1:2, :])
                bc = pe.tile([128, 3 * NCLS], F32)
                nc.tensor.matmul(bc[:, 0:NCLS], r32(ones_sb[:, :]), r32(crow2[0:1, 0:NCLS]),
                                 start=True, stop=True, skip_group_check=True)
                nc.tensor.matmul(bc[:, NCLS:2 * NCLS], r32(ones_sb[:, :]),
                                 r32(crow2[0:1, NCLS:2 * NCLS]),
                                 start=True, stop=True, skip_group_check=True)
                nc.tensor.matmul(bc[:, 2 * NCLS:3 * NCLS], r32(ones2[:, :]), r32(sq2[:, :]),
                                 start=True, stop=True, skip_group_check=True)
                c0b, c1b, csb = (bc[:, 0:NCLS], bc[:, NCLS:2 * NCLS],
                                 bc[:, 2 * NCLS:3 * NCLS])
                scall = se.tile([128, RB * NCLS], F32)
                for rb in range(RB):
                    g0 = gpct[:, rb * 2:rb * 2 + 1].bitcast(F32)
                    g1 = gpct[:, rb * 2 + 1:rb * 2 + 2].bitcast(F32)
                    sqg = se.tile([128, 2], F32, tag="sqg")
                    nc.vector.tensor_mul(sqg[:, :], gpct[:, rb * 2:(rb + 1) * 2],
                                         gpct[:, rb * 2:(rb + 1) * 2])
                    gsq = se.tile([128, 1], F32, tag="gsq")
                    nc.vector.reduce_sum(gsq[:, :], sqg[:, :], axis=mybir.AxisListType.X)
                    v1 = se.tile([128, NCLS], F32, tag="v1")
                    nc.vector.tensor_scalar(v1[:, :], c1b, g1, None, ALU.mult)
                    v2 = se.tile([128, NCLS], F32, tag="v2")
                    nc.vector.scalar_tensor_tensor(
                        v2[:, :], c0b, g0, v1[:, :], ALU.mult, ALU.add)
                    v3 = se.tile([128, NCLS], F32, tag="v3")
                    nc.vector.scalar_tensor_tensor(
                        v3[:, :], v2[:, :], -2.0, csb, ALU.mult, ALU.add)
                    v4 = se.tile([128, NCLS], F32, tag="v4")
                    nc.vector.tensor_scalar(
                        v4[:, :], v3[:, :], gsq[:, :], 0.0, ALU.add, ALU.max)
                    v5 = se.tile([128, NCLS], F32, tag="v5")
                    nc.scalar.activation(v5[:, :], v4[:, :], AF.Sqrt)
                    nc.vector.tensor_scalar_mul(
                        scall[:, rb * NCLS:(rb + 1) * NCLS], v5[:, :], -1.0)
                nc.sync.dma_start(
                    o_sc[:, :].rearrange("(rb p) c -> p rb c", p=128), scall[:, :])
    return nc


_CACHE = {}


def _get_nc(N, CORES, H, NCLS, K, CC, SEL):
    key = (N, CORES, H, NCLS, K, CC, SEL)
    if key not in _CACHE:
        _CACHE[key] = build(N, CORES, H, NCLS, K, CC, SEL)
    return _CACHE[key]


def round_fp32r(a):
    b = np.ascontiguousarray(a, np.float32).view(np.uint32)
    lsb = (b >> 12) & 1
    r = (b + 0x7FF + lsb) & 0xFFFFF000
    return r.view(np.float32)


def prep_inputs(x, ohm_labels, W, b, spars, CORES=8):
    N, H = x.shape
    NCLS = ohm_labels.shape[1]
    ROWS = N // CORES
    RB = ROWS // 128
    x = np.asarray(x, np.float32)
    ohm = np.asarray(ohm_labels, np.float32)
    W = np.asarray(W, np.float32)
    b = np.asarray(b, np.float32)
    u = (ohm.sum(1) == 0).astype(np.float32)
    xsq = (x * x).sum(1).astype(np.float32)
    h = (x @ W.T + b).astype(np.float32)
    fc = np.float32(1.0 / max(float(u.sum()), 1.0))
    xt = np.ascontiguousarray(x.T)
    aug1 = (-xsq)[None, :].astype(np.float32)
    aug2 = (-xsq - BIG * (1.0 - u))[None, :].astype(np.float32)
    rh12 = np.concatenate(
        [ohm.T, (fc * (1.0 - u))[None, :], (fc * u)[None, :]], 0).astype(np.float32)
    counts = ohm.sum(0)
    inv = np.where(counts > 0, 1.0 / np.maximum(counts, 1.0), 0.0).astype(np.float32)
    invc2 = np.ascontiguousarray(np.stack([inv, inv], 0))

    def perm(v):  # [ROWS,...] -> [128, RB, ...] -> [128, RB*...]
        r = v.reshape(RB, 128, -1).transpose(1, 0, 2).reshape(128, -1)
        return np.ascontiguousarray(r.astype(np.float32))

    in_maps = []
    for c in range(CORES):
        rs = slice(c * ROWS, (c + 1) * ROWS)
        bias0 = xsq[rs] * (1.0 + INFL) + 1e-6 + GUARD
        hl = h[rs]
        aggl = ohm[rs].T @ hl                     # [NCLS, 2]
        t1 = (1.0 - u[rs]) @ hl                   # [2]
        t2 = u[rs] @ hl                           # [2]
        bghv = (ohm @ aggl + fc * (np.outer(u, t1) + np.outer(1.0 - u, t2)))
        in_maps.append({
            "bgh": np.ascontiguousarray(bghv.T.astype(np.float32)),
            "xt": round_fp32r(xt), "aug1": round_fp32r(aug1),
            "aug2": round_fp32r(aug2), "rh12": rh12.astype(ml_dtypes.bfloat16),
            "invc2": invc2,
            "x2t": round_fp32r(2.0 * x[rs].T),
            "lh12": np.concatenate(
                [ohm[rs].T, u[rs][None, :], (1.0 - u[rs])[None, :]],
                0).astype(ml_dtypes.bfloat16),
            "bias0": perm(bias0),
            "bias2": perm(bias0 + BIG * (1.0 - u[rs])),
            "uloc": perm(u[rs]),
            "hloc": round_fp32r(perm(h[rs])),
            "ohml": round_fp32r(perm(ohm[rs])),
        })
    return in_maps


def kernel(x, ohm_labels, W, b, spars):
    N, H = np.asarray(x).shape
    NCLS = np.asarray(ohm_labels).shape[1]
    CORES = 8
    K = int(spars)
    nc = _get_nc(N, CORES, H, NCLS, K, 512, 512)
    in_maps = prep_inputs(x, ohm_labels, W, b, spars, CORES)
    res = bass_utils.run_bass_kernel_spmd(nc, in_maps, core_ids=list(range(CORES)))
    outs = res.results
    adj = np.concatenate([o["o_adj"] for o in outs], 0).astype(np.float32)
    g = np.concatenate([o["o_g"] for o in outs], 0)
    sc = np.concatenate([o["o_sc"] for o in outs], 0)
    return sc, g, adj


# revision 35
# speedup vs baseline: 1.2671x; 1.0387x over previous
"""AdaGAE processor kernel for 8 Trainium2 NeuronCores.

Row-shards the n dimension across 8 cores. Each core computes its
[n/8, n] stripe of the adjacency kernel, plus partial g = adj @ h via
PE block-matmuls, with two small collectives (AllGather of per-row sort
stats, ReduceScatter of partial g, AllReduce of centroid aggregates).

Math notes (vs the reference):
 - a_i = 33rd-smallest distance in row i, b_i = sum of 32 smallest.
   Found via max8/match_replace hierarchical selection on s = 2G - |x_j|^2
   (row-wise ordering of s is exactly the reverse of d^2).
 - denom_i = (a_i+1e-10)*k - b_i + 1e-10;  s1_i = 1/(2*denom_i)
 - sym adagae (uu region) = relu(s1_i*(a_i' - d)) + relu(q_j*(a_j' - d)),
   q_j = u_j * s1_j  -> no transpose needed, only allgathered row vectors.
 - kernel = semantic + fc*(mixed region) + [uu]*adagae. Region masking is
   done with +BIG terms folded into the matmul aug row (known columns) and
   the sqrt bias (known rows), which push d' so high every relu dies.
 - d = sqrt(xsq_i*(1+2e-5)+1e-6 + xsq_j - 2*x_i.x_j): tiny consistent
   inflation guards sqrt(negative) on the diagonal without a clamp pass.
"""

import sys

if "/opt/trn_rl_repo" not in sys.path:
    sys.path.insert(0, "/opt/trn_rl_repo")

import numpy as np
import ml_dtypes

import concourse.bass as bass
import concourse.bacc as bacc
import concourse.mybir as mybir
from concourse import tile
from concourse import bass_utils


# --- workaround: this container's walrus rejects CTRL instructions with >1
# sync wait; split the kernel-tail drain's waits across single-wait drains. ---
from concourse.vector_clock import ScopedClock as _ScopedClock


_WAIT_LIM = 1


def _split_excess_waits(nc_):
    # this walrus build rejects instructions with >_WAIT_LIM sync waits;
    # carry the excess on engine NOPs inserted just before the instruction.
    f = nc_.m.functions[0]

    ws = nc_._ws_sem

    def make_nop(eng):
        nop = nc_.engines[eng].wait_ge(ws, 0)
        mi = nop.ins
        for b2 in f.blocks:
            il2 = b2.instructions
            if il2 and il2[-1].name == mi.name:
                il2.pop()
                b2.instructions = il2
                return mi
        raise RuntimeError("nop not found in any block")

    for bb in f.blocks:
        il = list(bb.instructions)
        out = []
        changed = False
        for inst in il:
            si = getattr(inst, "sync_info", None)
            waits = list(si.on_wait) if (si is not None and si.on_wait) else []
            if len(waits) > _WAIT_LIM:
                changed = True
                extra, keep = waits[:-_WAIT_LIM], waits[-_WAIT_LIM:]
                for i in range(0, len(extra), _WAIT_LIM):
                    mi = make_nop(inst.engine)
                    mi.sync_info = mybir.SyncInfo(
                        on_wait=extra[i:i + _WAIT_LIM], on_update=[])
                    out.append(mi)
                inst.sync_info = mybir.SyncInfo(
                    on_wait=keep, on_update=list(si.on_update or []))
            out.append(inst)
        if changed:
            bb.instructions = out


def _split_drain_and_barrier(self, tick_clock, wait_clock):
    nc_ = self.nc
    drain_inst = nc_.sync.drain()
    wait_clock.add_sem_waits(
        drain_inst.ins, _ScopedClock({None: tick_clock.global_clock}))
    mi = drain_inst.ins
    si = mi.sync_info
    if si is not None and si.on_wait and len(si.on_wait) > 1:
        waits = list(si.on_wait)
        mi.sync_info = mybir.SyncInfo(on_wait=[waits[0]], on_update=list(si.on_update or []))
        for w in waits[1:]:
            d2 = nc_.sync.drain()
            d2.ins.sync_info = mybir.SyncInfo(on_wait=[w], on_update=[])
    _split_excess_waits(nc_)
    nc_.all_engine_barrier()
    assert self.sems is not None
    popped = nc_._tile_sem_poison_stack.pop()
    assert popped is self._sem_poison
    nc_.clear_and_free_semaphores(list(self.sems.allocated().values()))
    nc_.all_engine_barrier()


tile.TileContext._drain_and_barrier = _split_drain_and_barrier

F32 = mybir.dt.float32
F32R = mybir.dt.float32r
BF16 = mybir.dt.bfloat16
AF = mybir.ActivationFunctionType
ALU = mybir.AluOpType

BIG = 1.0e6
INFL = 2.0e-5  # relative inflation of xsq_i in d^2 (diag sqrt guard)
GUARD = 0.25  # absolute d^2 floor: covers fp32r matmul rounding on the diagonal
NEG = -3.0e38


def r32(ap):
    return ap.bitcast(F32R)


def build(N=8192, CORES=8, H=128, NCLS=10, K=32, CC=512, SEL=256):
    ROWS = N // CORES
    RB = ROWS // 128
    NCC = N // CC
    NSEL = CC // SEL
    NCAND = NCC * NSEL * 8
    NR = (K + 8) // 8  # selection rounds (5 for K=32 -> top-40)
    NTOP = NR * 8
    assert NTOP > K
    RG = [list(range(CORES))]

    nc = bass.Bass("TRN2", target_bir_lowering=False, num_devices=CORES)
    _ws_ctx = nc.semaphore("waitsplit")
    nc._ws_sem = _ws_ctx.__enter__()

    # ---------------- I/O ----------------
    xt = nc.dram_tensor("xt", [H, N], F32R, kind="ExternalInput")
    x2t = nc.dram_tensor("x2t", [H, ROWS], F32R, kind="ExternalInput")
    aug1 = nc.dram_tensor("aug1", [1, N], F32R, kind="ExternalInput")
    aug2 = nc.dram_tensor("aug2", [1, N], F32R, kind="ExternalInput")
    rh12 = nc.dram_tensor("rh12", [12, N], BF16, kind="ExternalInput")
    lh12 = nc.dram_tensor("lh12", [12, ROWS], BF16, kind="ExternalInput")
    bias0 = nc.dram_tensor("bias0", [128, RB], F32, kind="ExternalInput")
    bias2 = nc.dram_tensor("bias2", [128, RB], F32, kind="ExternalInput")
    uloc = nc.dram_tensor("uloc", [128, RB], F32, kind="ExternalInput")
    hloc = nc.dram_tensor("hloc", [128, RB * 2], F32R, kind="ExternalInput")
    ohml = nc.dram_tensor("ohml", [128, RB * NCLS], F32R, kind="ExternalInput")
    invc2 = nc.dram_tensor("invc2", [2, NCLS], F32, kind="ExternalInput")
    bgh = nc.dram_tensor("bgh", [2, N], F32, kind="ExternalInput")

    o_adj = nc.dram_tensor("o_adj", [ROWS, N], BF16, kind="ExternalOutput")
    o_g = nc.dram_tensor("o_g", [ROWS, 2], F32, kind="ExternalOutput")
    o_sc = nc.dram_tensor("o_sc", [ROWS, NCLS], F32, kind="ExternalOutput")

    with tile.TileContext(nc) as tc:
        with (
            tc.tile_pool(name="const", bufs=1) as cpool,
            tc.tile_pool(name="dram", bufs=1, space="DRAM") as dpool,
        ):
            # persistent SBUF tiles
            xt_sb = cpool.tile([H, N], F32R)
            x2t_sb = cpool.tile([H, ROWS], F32R)
            lh12_sb = cpool.tile([12, ROWS], BF16)
            bias0_sb = cpool.tile([128, RB], F32)
            bias2_sb = cpool.tile([128, RB], F32)
            uloc_sb = cpool.tile([128, RB], F32)
            hloc_sb = cpool.tile([128, RB * 2], F32R)
            ohml_sb = cpool.tile([128, RB * NCLS], F32R)
            invc2_sb = cpool.tile([2, NCLS], F32)
            ones_f32 = cpool.tile([1, 128], F32)
            ones_sb = cpool.tile([1, 128], F32R)
            # per-row stats (phase1 -> phase2)
            a1b_sb = cpool.tile([128, RB], F32)  # (a+eps)*s1
            ns1_sb = cpool.tile([128, RB], F32)  # -s1
            q_sb = cpool.tile([128, RB], F32R)  # u*s1
            qa_sb = cpool.tile([128, RB], F32R)  # (a+eps)*u*s1

            for t, src in [
                (xt_sb, xt), (x2t_sb, x2t),
                (lh12_sb, lh12), (bias0_sb, bias0),
                (bias2_sb, bias2), (uloc_sb, uloc), (hloc_sb, hloc),
                (ohml_sb, ohml), (invc2_sb, invc2),
            ]:
                nc.sync.dma_start(t[:, :], src[:, :])
            nc.vector.memset(ones_f32[:, :], 1.0)
            nc.vector.tensor_copy(ones_sb[:, :], ones_f32[:, :])

            # DRAM bounce buffers for collectives
            gb_in = dpool.tile([1, 2 * ROWS], F32R)
            gb = dpool.tile([CORES, 2 * ROWS], F32R)
            gpart0 = dpool.tile([1, N], F32)
            gpart1 = dpool.tile([1, N], F32)
            grs0 = dpool.tile([1, ROWS], F32)
            grs1 = dpool.tile([1, ROWS], F32)
            aggd = dpool.tile([NCLS, 2], F32)
            aggf = dpool.tile([NCLS, 2], F32)

            # ================= PHASE 1: row stats =================
            with (
                tc.tile_pool(name="p1psum", bufs=2, space="PSUM") as pp1,
                tc.tile_pool(name="p1sb", bufs=3) as sp1,
                tc.tile_pool(name="p1small", bufs=3) as sm1,
            ):
                for rb in range(RB):
                    cands = sp1.tile([128, NCAND], F32, tag="cands")
                    for cc in range(NCC):
                        ps = pp1.tile([128, CC], F32, tag="ps1")
                        a1row = sp1.tile([1, CC], F32R, tag="a1row")
                        nc.sync.dma_start(a1row[:, :], aug1[:, cc * CC:(cc + 1) * CC])
                        nc.tensor.matmul(
                            ps[:, :], r32(x2t_sb[:, rb * 128:(rb + 1) * 128]),
                            r32(xt_sb[:, cc * CC:(cc + 1) * CC]),
                            start=True, stop=False)
                        nc.tensor.matmul(
                            ps[:, :], r32(ones_sb[:, :]), r32(a1row[:, :]),
                            start=False, stop=True)
                        sch = sp1.tile([128, CC], F32, tag="sch")
                        nc.scalar.copy(sch[:, :], ps[:, :])
                        for sl in range(NSEL):
                            nc.vector.max(
                                cands[:, (cc * NSEL + sl) * 8:(cc * NSEL + sl + 1) * 8],
                                sch[:, sl * SEL:(sl + 1) * SEL])
                    # stage B: top-NTOP of candidates
                    maxima = sm1.tile([128, NTOP], F32, tag="maxima")
                    work = cands
                    for r in range(NR):
                        nc.vector.max(maxima[:, r * 8:(r + 1) * 8], work[:, :])
                        if r < NR - 1:
                            nwork = sp1.tile([128, NCAND], F32, tag="cands")
                            nc.vector.match_replace(
                                nwork[:, :], maxima[:, r * 8:(r + 1) * 8],
                                work[:, :], NEG)
                            work = nwork
                    # stats
                    d2t = sm1.tile([128, NTOP], F32, tag="d2t")
                    nc.vector.tensor_scalar(
                        d2t[:, :], maxima[:, :], -1.0, bias0_sb[:, rb:rb + 1],
                        ALU.mult, ALU.add)
                    d40 = sm1.tile([128, NTOP], F32, tag="d40")
                    nc.scalar.activation(d40[:, :], d2t[:, :], AF.Sqrt)
                    bsum = sm1.tile([128, 1], F32, tag="bsum")
                    nc.vector.reduce_sum(bsum[:, :], d40[:, 0:K], axis=mybir.AxisListType.X)
                    akt = sm1.tile([128, 1], F32, tag="akt")
                    nc.vector.tensor_scalar(
                        akt[:, :], d40[:, K:K + 1], float(K), (K + 1) * 1e-10,
                        ALU.mult, ALU.add)
                    den = sm1.tile([128, 1], F32, tag="den")
                    nc.vector.tensor_sub(den[:, :], akt[:, :], bsum[:, :])
                    dd2 = sm1.tile([128, 1], F32, tag="dd2")
                    nc.vector.tensor_scalar_mul(dd2[:, :], den[:, :], 2.0)
                    s1t = sm1.tile([128, 1], F32, tag="s1t")
                    nc.vector.reciprocal(s1t[:, :], dd2[:, :])
                    aep = sm1.tile([128, 1], F32, tag="aep")
                    nc.vector.tensor_scalar_add(aep[:, :], d40[:, K:K + 1], 1e-10)
                    nc.vector.tensor_mul(a1b_sb[:, rb:rb + 1], aep[:, :], s1t[:, :])
                    nc.vector.tensor_scalar_mul(ns1_sb[:, rb:rb + 1], s1t[:, :], -1.0)
                    nc.vector.tensor_mul(q_sb[:, rb:rb + 1], s1t[:, :], uloc_sb[:, rb:rb + 1])
                    nc.vector.tensor_mul(qa_sb[:, rb:rb + 1], aep[:, :], q_sb[:, rb:rb + 1])

            # ship q/qa to DRAM bounce, allgather
            qv = gb_in[0, 0:ROWS].rearrange("(rb p) -> p rb", p=128)
            qav = gb_in[0, ROWS:2 * ROWS].rearrange("(rb p) -> p rb", p=128)
            nc.sync.dma_start(qv, q_sb[:, :])
            nc.sync.dma_start(qav, qa_sb[:, :])
            nc.gpsimd.collective_compute(
                "AllGather", ALU.bypass, replica_groups=RG,
                ins=[gb_in[:, :].opt()], outs=[gb[:, :].opt()])

            # ================= PHASE 2: adjacency stripe =================
            CC2 = min(int(__import__('os').environ.get('KCC2', '2048')), N)
            NCC2 = N // CC2
            SUB = CC2 // 512
            with (
                tc.tile_pool(name="p2ps_s", bufs=2, space="PSUM") as pps,
                tc.tile_pool(name="p2ps_b", bufs=(1 if CC2 >= 2048 else 2), space="PSUM") as ppb,
                tc.tile_pool(name="p2ps_g", bufs=1, space="PSUM") as ppg,
                tc.tile_pool(name="p2sb", bufs=2) as sp2,
                tc.tile_pool(name="p2row", bufs=2) as rp2,
            ):
                for cc in range(NCC2):
                    qbc = sp2.tile([128, CC2], F32, tag="qbc")
                    qabc = sp2.tile([128, CC2], F32, tag="qabc")
                    pos = 0
                    while pos < CC2:
                        j = cc * CC2 + pos
                        dev, off = j // ROWS, j % ROWS
                        L = min(ROWS - off, CC2 - pos)
                        nc.sync.dma_start(
                            qbc[:, pos:pos + L],
                            gb[dev, off:off + L].bitcast(F32).partition_broadcast(128))
                        nc.sync.dma_start(
                            qabc[:, pos:pos + L],
                            gb[dev, ROWS + off:ROWS + off + L].bitcast(F32).partition_broadcast(128))
                        pos += L
                    a2row = rp2.tile([1, CC2], F32R, tag="a2row")
                    r12t = rp2.tile([12, CC2], BF16, tag="r12t")
                    nc.sync.dma_start(a2row[:, :], aug2[:, cc * CC2:(cc + 1) * CC2])
                    nc.sync.dma_start(r12t[:, :], rh12[:, cc * CC2:(cc + 1) * CC2])
                    gpss = [ppg.tile([2, 512], F32, tag=f"gps{s}", name=f"gps{s}_{cc}")
                            for s in range(SUB)]

                    SKEW = 2
                    dps = {}

                    def stage_a(rb):
                        dp = sp2.tile([128, CC2], F32, tag="dp", bufs=SKEW + 2,
                                      name=f"dp_{cc}_{rb}")
                        for sb in range(SUB):
                            ps = pps.tile([128, 512], F32, tag="ps2", bufs=2,
                                          name=f"ps_{cc}_{rb}_{sb}")
                            c0 = cc * CC2 + sb * 512
                            nc.tensor.matmul(
                                ps[:, :], r32(x2t_sb[:, rb * 128:(rb + 1) * 128]),
                                r32(xt_sb[:, c0:c0 + 512]),
                                start=True, stop=False)
                            nc.tensor.matmul(
                                ps[:, :], r32(ones_sb[:, :]),
                                r32(a2row[:, sb * 512:(sb + 1) * 512]),
                                start=False, stop=True)
                            nc.scalar.activation(
                                dp[:, sb * 512:(sb + 1) * 512], ps[:, :], AF.Sqrt,
                                bias=bias2_sb[:, rb:rb + 1], scale=-1.0)
                        dps[rb] = dp

                    def stage_b(rb):
                        dp = dps.pop(rb)
                        a1 = sp2.tile([128, CC2], F32, tag="a1", name=f"a1_{cc}_{rb}")
                        nc.scalar.activation(a1[:, :], dp[:, :], AF.Relu,
                                             bias=a1b_sb[:, rb:rb + 1],
                                             scale=ns1_sb[:, rb:rb + 1])
                        mt = sp2.tile([128, CC2], F32R, tag="mt", name=f"mt_{cc}_{rb}")
                        nc.gpsimd.tensor_tensor(mt[:, :], dp[:, :], qbc[:, :],
                                                op=ALU.mult)
                        nc.vector.scalar_tensor_tensor(
                            mt[:, :], mt[:, :], 0.0, qabc[:, :], ALU.add, ALU.subtract)
                        nc.vector.scalar_tensor_tensor(
                            mt[:, :], mt[:, :], 0.0, a1[:, :], ALU.min, ALU.subtract)
                        adjt = sp2.tile([128, CC2], BF16, tag="adjt",
                                        name=f"adjt_{cc}_{rb}")
                        for sb in range(SUB):
                            bps = ppb.tile([128, 512], F32, tag="bps", bufs=1,
                                           name=f"bps_{cc}_{rb}_{sb}")
                            nc.tensor.matmul(
                                bps[:, :], lh12_sb[:, rb * 128:(rb + 1) * 128],
                                r12t[:, sb * 512:(sb + 1) * 512],
                                start=True, stop=True)
                            nc.vector.tensor_sub(
                                adjt[:, sb * 512:(sb + 1) * 512], bps[:, :],
                                mt[:, sb * 512:(sb + 1) * 512])
                            nc.tensor.matmul(
                                gpss[sb][:, :],
                                r32(hloc_sb[:, rb * 2:(rb + 1) * 2]),
                                mt[:, sb * 512:(sb + 1) * 512],
                                start=(rb == 0), stop=(rb == RB - 1))
                        nc.sync.dma_start(
                            o_adj[rb * 128:(rb + 1) * 128, cc * CC2:(cc + 1) * CC2],
                            adjt[:, :])

                    for it in range(RB + SKEW):
                        if it < RB:
                            stage_a(it)
                        if it >= SKEW:
                            stage_b(it - SKEW)

                    bght = rp2.tile([2, CC2], F32, tag="bght", bufs=1)
                    nc.sync.dma_start(bght[:, :], bgh[:, cc * CC2:(cc + 1) * CC2])
                    gstage = rp2.tile([2, CC2], F32, tag="gstage", bufs=1)
                    for sb in range(SUB):
                        nc.vector.tensor_sub(gstage[:, sb * 512:(sb + 1) * 512],
                                             bght[:, sb * 512:(sb + 1) * 512],
                                             gpss[sb][:, :])
                    nc.sync.dma_start(gpart0[0, cc * CC2:(cc + 1) * CC2], gstage[0:1, :])
                    nc.sync.dma_start(gpart1[0, cc * CC2:(cc + 1) * CC2], gstage[1:2, :])

            # reduce-scatter partial g to local rows (per component)
            nc.gpsimd.collective_compute(
                "ReduceScatter", ALU.add, replica_groups=RG,
                ins=[gpart0[:, :].opt()], outs=[grs0[:, :].opt()])
            nc.gpsimd.collective_compute(
                "ReduceScatter", ALU.add, replica_groups=RG,
                ins=[gpart1[:, :].opt()], outs=[grs1[:, :].opt()])

            # ================= EPILOGUE =================
            with (
                tc.tile_pool(name="e_ps", bufs=1, space="PSUM") as pe,
                tc.tile_pool(name="e_sb", bufs=1) as se,
            ):
                gpc = se.tile([128, RB * 2], F32)
                nc.sync.dma_start(
                    gpc[:, :].rearrange("p (rb c) -> c p rb", c=2)[0],
                    grs0[0, :].rearrange("(rb p) -> p rb", p=128))
                nc.sync.dma_start(
                    gpc[:, :].rearrange("p (rb c) -> c p rb", c=2)[1],
                    grs1[0, :].rearrange("(rb p) -> p rb", p=128))
                gpct = se.tile([128, RB * 2], F32R)
                nc.scalar.activation(gpct[:, :], gpc[:, :], AF.Tanh)
                nc.sync.dma_start(
                    o_g[:, :].rearrange("(rb p) c -> p rb c", p=128),
                    gpct[:, :].bitcast(F32))
                # centroid partial agg = ohm_loc.T @ tanh(g_loc)
                aggps = pe.tile([NCLS, 2], F32)
                for rb in range(RB):
                    nc.tensor.matmul(
                        aggps[:, :], r32(ohml_sb[:, rb * NCLS:(rb + 1) * NCLS]),
                        r32(gpct[:, rb * 2:(rb + 1) * 2]),
                        start=(rb == 0), stop=(rb == RB - 1))
                aggsb = se.tile([NCLS, 2], F32)
                nc.vector.tensor_copy(aggsb[:, :], aggps[:, :])
                nc.sync.dma_start(aggd[:, :], aggsb[:, :])
                nc.gpsimd.collective_compute(
                    "AllReduce", ALU.add, replica_groups=RG,
                    ins=[aggd[:, :].opt()], outs=[aggf[:, :].opt()])
                aggt = se.tile([2, NCLS], F32)
                nc.sync.dma_start(aggt[:, :], aggf[:, :].rearrange("m c -> c m"))
                ctrt = se.tile([2, NCLS], F32R)
                nc.vector.tensor_mul(ctrt[:, :], aggt[:, :], invc2_sb[:, :])
                sq2 = se.tile([2, NCLS], F32R)
                nc.vector.tensor_mul(sq2[:, :], ctrt[:, :], ctrt[:, :])
                ones2f = se.tile([2, 128], F32)
                ones2 = se.tile([2, 128], F32R)
                nc.vector.memset(ones2f[:, :], 1.0)
                nc.vector.tensor_copy(ones2[:, :], ones2f[:, :])
                crow2 = se.tile([1, 2 * NCLS], F32R)
                nc.sync.dma_start(crow2[0:1, 0:NCLS], ctrt[0:1, :])
                nc.sync.dma_start(crow2[0:1, NCLS:2 * NCLS], ctrt[1:2, :])
                bc = pe.tile([128, 3 * NCLS], F32)
                nc.tensor.matmul(bc[:, 0:NCLS], r32(ones_sb[:, :]), r32(crow2[0:1, 0:NCLS]),
                                 start=True, stop=True, skip_group_check=True)
                nc.tensor.matmul(bc[:, NCLS:2 * NCLS], r32(ones_sb[:, :]),
                                 r32(crow2[0:1, NCLS:2 * NCLS]),
                                 start=True, stop=True, skip_group_check=True)
                nc.tensor.matmul(bc[:, 2 * NCLS:3 * NCLS], r32(ones2[:, :]), r32(sq2[:, :]),
                                 start=True, stop=True, skip_group_check=True)
                c0b, c1b, csb = (bc[:, 0:NCLS], bc[:, NCLS:2 * NCLS],
                                 bc[:, 2 * NCLS:3 * NCLS])
                scall = se.tile([128, RB * NCLS], F32)
                for rb in range(RB):
                    g0 = gpct[:, rb * 2:rb * 2 + 1].bitcast(F32)
                    g1 = gpct[:, rb * 2 + 1:rb * 2 + 2].bitcast(F32)
                    sqg = se.tile([128, 2], F32, tag="sqg")
                    nc.vector.tensor_mul(sqg[:, :], gpct[:, rb * 2:(rb + 1) * 2],
                                         gpct[:, rb * 2:(rb + 1) * 2])
                    gsq = se.tile([128, 1], F32, tag="gsq")
                    nc.vector.reduce_sum(gsq[:, :], sqg[:, :], axis=mybir.AxisListType.X)
                    v1 = se.tile([128, NCLS], F32, tag="v1")
                    nc.vector.tensor_scalar(v1[:, :], c1b, g1, None, ALU.mult)
                    v2 = se.tile([128, NCLS], F32, tag="v2")
                    nc.vector.scalar_tensor_tensor(
                        v2[:, :], c0b, g0, v1[:, :], ALU.mult, ALU.add)
                    v3 = se.tile([128, NCLS], F32, tag="v3")
                    nc.vector.scalar_tensor_tensor(
                        v3[:, :], v2[:, :], -2.0, csb, ALU.mult, ALU.add)
                    v4 = se.tile([128, NCLS], F32, tag="v4")
                    nc.vector.tensor_scalar(
                        v4[:, :], v3[:, :], gsq[:, :], 0.0, ALU.add, ALU.max)
                    v5 = se.tile([128, NCLS], F32, tag="v5")
                    nc.scalar.activation(v5[:, :], v4[:, :], AF.Sqrt)
                    nc.vector.tensor_scalar_mul(
                        scall[:, rb * NCLS:(rb + 1) * NCLS], v5[:, :], -1.0)
                nc.sync.dma_start(
                    o_sc[:, :].rearrange("(rb p) c -> p rb c", p=128), scall[:, :])
    return nc


_CACHE = {}


def _get_nc(N, CORES, H, NCLS, K, CC, SEL):
    key = (N, CORES, H, NCLS, K, CC, SEL)
    if key not in _CACHE:
        _CACHE[key] = build(N, CORES, H, NCLS, K, CC, SEL)
    return _CACHE[key]


def round_fp32r(a):
    b = np.ascontiguousarray(a, np.float32).view(np.uint32)
    lsb = (b >> 12) & 1
    r = (b + 0x7FF + lsb) & 0xFFFFF000
    return r.view(np.float32)


def prep_inputs(x, ohm_labels, W, b, spars, CORES=8):
    N, H = x.shape
    NCLS = ohm_labels.shape[1]
    ROWS = N // CORES
    RB = ROWS // 128
    x = np.asarray(x, np.float32)
    ohm = np.asarray(ohm_labels, np.float32)
    W = np.asarray(W, np.float32)
    b = np.asarray(b, np.float32)
    u = (ohm.sum(1) == 0).astype(np.float32)
    xsq = (x * x).sum(1).astype(np.float32)
    h = (x @ W.T + b).astype(np.float32)
    fc = np.float32(1.0 / max(float(u.sum()), 1.0))
    xt = np.ascontiguousarray(x.T)
    aug1 = (-xsq)[None, :].astype(np.float32)
    aug2 = (-xsq - BIG * (1.0 - u))[None, :].astype(np.float32)
    rh12 = np.concatenate(
        [ohm.T, (fc * (1.0 - u))[None, :], (fc * u)[None, :]], 0).astype(np.float32)
    counts = ohm.sum(0)
    inv = np.where(counts > 0, 1.0 / np.maximum(counts, 1.0), 0.0).astype(np.float32)
    invc2 = np.ascontiguousarray(np.stack([inv, inv], 0))

    def perm(v):  # [ROWS,...] -> [128, RB, ...] -> [128, RB*...]
        r = v.reshape(RB, 128, -1).transpose(1, 0, 2).reshape(128, -1)
        return np.ascontiguousarray(r.astype(np.float32))

    in_maps = []
    for c in range(CORES):
        rs = slice(c * ROWS, (c + 1) * ROWS)
        bias0 = xsq[rs] * (1.0 + INFL) + 1e-6 + GUARD
        hl = h[rs]
        aggl = ohm[rs].T @ hl                     # [NCLS, 2]
        t1 = (1.0 - u[rs]) @ hl                   # [2]
        t2 = u[rs] @ hl                           # [2]
        bghv = (ohm @ aggl + fc * (np.outer(u, t1) + np.outer(1.0 - u, t2)))
        in_maps.append({
            "bgh": np.ascontiguousarray(bghv.T.astype(np.float32)),
            "xt": round_fp32r(xt), "aug1": round_fp32r(aug1),
            "aug2": round_fp32r(aug2), "rh12": rh12.astype(ml_dtypes.bfloat16),
            "invc2": invc2,
            "x2t": round_fp32r(2.0 * x[rs].T),
            "lh12": np.concatenate(
                [ohm[rs].T, u[rs][None, :], (1.0 - u[rs])[None, :]],
                0).astype(ml_dtypes.bfloat16),
            "bias0": perm(bias0),
            "bias2": perm(bias0 + BIG * (1.0 - u[rs])),
            "uloc": perm(u[rs]),
            "hloc": round_fp32r(perm(h[rs])),
            "ohml": round_fp32r(perm(ohm[rs])),
        })
    return in_maps


def kernel(x, ohm_labels, W, b, spars):
    N, H = np.asarray(x).shape
    NCLS = np.asarray(ohm_labels).shape[1]
    CORES = 8
    K = int(spars)
    nc = _get_nc(N, CORES, H, NCLS, K, 512, 512)
    in_maps = prep_inputs(x, ohm_labels, W, b, spars, CORES)
    res = bass_utils.run_bass_kernel_spmd(nc, in_maps, core_ids=list(range(CORES)))
    outs = res.results
    adj = np.concatenate([o["o_adj"] for o in outs], 0).astype(np.float32)
    g = np.concatenate([o["o_g"] for o in outs], 0)
    sc = np.concatenate([o["o_sc"] for o in outs], 0)
    return sc, g, adj


# revision 36
# speedup vs baseline: 1.3257x; 1.0463x over previous
"""AdaGAE processor kernel for 8 Trainium2 NeuronCores.

Row-shards the n dimension across 8 cores. Each core computes its
[n/8, n] stripe of the adjacency kernel, plus partial g = adj @ h via
PE block-matmuls, with two small collectives (AllGather of per-row sort
stats, ReduceScatter of partial g, AllReduce of centroid aggregates).

Math notes (vs the reference):
 - a_i = 33rd-smallest distance in row i, b_i = sum of 32 smallest.
   Found via max8/match_replace hierarchical selection on s = 2G - |x_j|^2
   (row-wise ordering of s is exactly the reverse of d^2).
 - denom_i = (a_i+1e-10)*k - b_i + 1e-10;  s1_i = 1/(2*denom_i)
 - sym adagae (uu region) = relu(s1_i*(a_i' - d)) + relu(q_j*(a_j' - d)),
   q_j = u_j * s1_j  -> no transpose needed, only allgathered row vectors.
 - kernel = semantic + fc*(mixed region) + [uu]*adagae. Region masking is
   done with +BIG terms folded into the matmul aug row (known columns) and
   the sqrt bias (known rows), which push d' so high every relu dies.
 - d = sqrt(xsq_i*(1+2e-5)+1e-6 + xsq_j - 2*x_i.x_j): tiny consistent
   inflation guards sqrt(negative) on the diagonal without a clamp pass.
"""

import sys

if "/opt/trn_rl_repo" not in sys.path:
    sys.path.insert(0, "/opt/trn_rl_repo")

import numpy as np
import ml_dtypes

import concourse.bass as bass
import concourse.bacc as bacc
import concourse.mybir as mybir
from concourse import tile
from concourse import bass_utils


# --- workaround: this container's walrus rejects CTRL instructions with >1
# sync wait; split the kernel-tail drain's waits across single-wait drains. ---
from concourse.vector_clock import ScopedClock as _ScopedClock


_WAIT_LIM = 1


def _split_excess_waits(nc_):
    # this walrus build rejects instructions with >_WAIT_LIM sync waits;
    # carry the excess on engine NOPs inserted just before the instruction.
    f = nc_.m.functions[0]

    ws = nc_._ws_sem

    def make_nop(eng):
        nop = nc_.engines[eng].wait_ge(ws, 0)
        mi = nop.ins
        for b2 in f.blocks:
            il2 = b2.instructions
            if il2 and il2[-1].name == mi.name:
                il2.pop()
                b2.instructions = il2
                return mi
        raise RuntimeError("nop not found in any block")

    for bb in f.blocks:
        il = list(bb.instructions)
        out = []
        changed = False
        for inst in il:
            si = getattr(inst, "sync_info", None)
            waits = list(si.on_wait) if (si is not None and si.on_wait) else []
            if len(waits) > _WAIT_LIM:
                changed = True
                extra, keep = waits[:-_WAIT_LIM], waits[-_WAIT_LIM:]
                for i in range(0, len(extra), _WAIT_LIM):
                    mi = make_nop(inst.engine)
                    mi.sync_info = mybir.SyncInfo(
                        on_wait=extra[i:i + _WAIT_LIM], on_update=[])
                    out.append(mi)
                inst.sync_info = mybir.SyncInfo(
                    on_wait=keep, on_update=list(si.on_update or []))
            out.append(inst)
        if changed:
            bb.instructions = out


def _split_drain_and_barrier(self, tick_clock, wait_clock):
    nc_ = self.nc
    drain_inst = nc_.sync.drain()
    wait_clock.add_sem_waits(
        drain_inst.ins, _ScopedClock({None: tick_clock.global_clock}))
    mi = drain_inst.ins
    si = mi.sync_info
    if si is not None and si.on_wait and len(si.on_wait) > 1:
        waits = list(si.on_wait)
        mi.sync_info = mybir.SyncInfo(on_wait=[waits[0]], on_update=list(si.on_update or []))
        for w in waits[1:]:
            d2 = nc_.sync.drain()
            d2.ins.sync_info = mybir.SyncInfo(on_wait=[w], on_update=[])
    _split_excess_waits(nc_)
    nc_.all_engine_barrier()
    assert self.sems is not None
    popped = nc_._tile_sem_poison_stack.pop()
    assert popped is self._sem_poison
    nc_.clear_and_free_semaphores(list(self.sems.allocated().values()))
    nc_.all_engine_barrier()


tile.TileContext._drain_and_barrier = _split_drain_and_barrier

F32 = mybir.dt.float32
F32R = mybir.dt.float32r
BF16 = mybir.dt.bfloat16
FP16 = mybir.dt.float16
AF = mybir.ActivationFunctionType
ALU = mybir.AluOpType

BIG = 1.0e6
INFL = 2.0e-5  # relative inflation of xsq_i in d^2 (diag sqrt guard)
GUARD = 0.35  # absolute d^2 floor: covers fp32r/fp16 rounding on the diagonal
CBIG = 3.0e4  # column-mask magnitude (must fit fp16)
NEG = -3.0e38


def r32(ap):
    return ap.bitcast(F32R)


def build(N=8192, CORES=8, H=128, NCLS=10, K=32, CC=512, SEL=256):
    ROWS = N // CORES
    RB = ROWS // 128
    NCC = N // CC
    NSEL = CC // SEL
    NCAND = NCC * NSEL * 8
    NR = (K + 8) // 8  # selection rounds (5 for K=32 -> top-40)
    NTOP = NR * 8
    assert NTOP > K
    RG = [list(range(CORES))]

    nc = bass.Bass("TRN2", target_bir_lowering=False, num_devices=CORES)
    _ws_ctx = nc.semaphore("waitsplit")
    nc._ws_sem = _ws_ctx.__enter__()

    # ---------------- I/O ----------------
    xt = nc.dram_tensor("xt", [H, N], F32R, kind="ExternalInput")
    x2t = nc.dram_tensor("x2t", [H, ROWS], F32R, kind="ExternalInput")
    aug1 = nc.dram_tensor("aug1", [1, N], FP16, kind="ExternalInput")
    aug2 = nc.dram_tensor("aug2", [2, N], FP16, kind="ExternalInput")
    rh12 = nc.dram_tensor("rh12", [12, N], BF16, kind="ExternalInput")
    lh12 = nc.dram_tensor("lh12", [12, ROWS], BF16, kind="ExternalInput")
    bias0 = nc.dram_tensor("bias0", [128, RB], F32, kind="ExternalInput")
    bias2 = nc.dram_tensor("bias2", [128, RB], F32, kind="ExternalInput")
    uloc = nc.dram_tensor("uloc", [128, RB], F32, kind="ExternalInput")
    hloc = nc.dram_tensor("hloc", [128, RB * 2], F32R, kind="ExternalInput")
    ohml = nc.dram_tensor("ohml", [128, RB * NCLS], F32R, kind="ExternalInput")
    invc2 = nc.dram_tensor("invc2", [2, NCLS], F32, kind="ExternalInput")
    bgh = nc.dram_tensor("bgh", [2, N], F32, kind="ExternalInput")
    o1h = nc.dram_tensor("o1h", [1, 128], FP16, kind="ExternalInput")
    o2h = nc.dram_tensor("o2h", [2, 128], FP16, kind="ExternalInput")

    o_adj = nc.dram_tensor("o_adj", [ROWS, N], BF16, kind="ExternalOutput")
    o_g = nc.dram_tensor("o_g", [ROWS, 2], F32, kind="ExternalOutput")
    o_sc = nc.dram_tensor("o_sc", [ROWS, NCLS], F32, kind="ExternalOutput")

    with tile.TileContext(nc) as tc:
        with (
            tc.tile_pool(name="const", bufs=1) as cpool,
            tc.tile_pool(name="dram", bufs=1, space="DRAM") as dpool,
        ):
            # persistent SBUF tiles
            xt_sb = cpool.tile([H, N], F32R)
            x2t_sb = cpool.tile([H, ROWS], F32R)
            lh12_sb = cpool.tile([12, ROWS], BF16)
            bias0_sb = cpool.tile([128, RB], F32)
            bias2_sb = cpool.tile([128, RB], F32)
            uloc_sb = cpool.tile([128, RB], F32)
            hloc_sb = cpool.tile([128, RB * 2], F32R)
            ohml_sb = cpool.tile([128, RB * NCLS], F32R)
            invc2_sb = cpool.tile([2, NCLS], F32)
            ones_f32 = cpool.tile([1, 128], F32)
            ones_sb = cpool.tile([1, 128], F32R)
            o1h_sb = cpool.tile([1, 128], FP16)
            o2h_sb = cpool.tile([2, 128], FP16)
            # per-row stats (phase1 -> phase2)
            a1b_sb = cpool.tile([128, RB], F32)  # (a+eps)*s1
            ns1_sb = cpool.tile([128, RB], F32)  # -s1
            q_sb = cpool.tile([128, RB], F32R)  # u*s1
            qa_sb = cpool.tile([128, RB], F32R)  # (a+eps)*u*s1

            for t, src in [
                (xt_sb, xt), (x2t_sb, x2t),
                (lh12_sb, lh12), (bias0_sb, bias0),
                (bias2_sb, bias2), (uloc_sb, uloc), (hloc_sb, hloc),
                (ohml_sb, ohml), (invc2_sb, invc2),
                (o1h_sb, o1h), (o2h_sb, o2h),
            ]:
                nc.sync.dma_start(t[:, :], src[:, :])
            nc.vector.memset(ones_f32[:, :], 1.0)
            nc.vector.tensor_copy(ones_sb[:, :], ones_f32[:, :])

            # DRAM bounce buffers for collectives
            gb_in = dpool.tile([1, 2 * ROWS], F32R)
            gb = dpool.tile([CORES, 2 * ROWS], F32R)
            gpart0 = dpool.tile([1, N], F32)
            gpart1 = dpool.tile([1, N], F32)
            grs0 = dpool.tile([1, ROWS], F32)
            grs1 = dpool.tile([1, ROWS], F32)
            aggd = dpool.tile([NCLS, 2], F32)
            aggf = dpool.tile([NCLS, 2], F32)

            # ================= PHASE 1: row stats =================
            with (
                tc.tile_pool(name="p1psum", bufs=2, space="PSUM") as pp1,
                tc.tile_pool(name="p1sb", bufs=3) as sp1,
                tc.tile_pool(name="p1small", bufs=3) as sm1,
            ):
                for rb in range(RB):
                    cands = sp1.tile([128, NCAND], F32, tag="cands")
                    for cc in range(NCC):
                        ps = pp1.tile([128, CC], F32, tag="ps1")
                        a1row = sp1.tile([1, CC], FP16, tag="a1row")
                        nc.sync.dma_start(a1row[:, :], aug1[:, cc * CC:(cc + 1) * CC])
                        nc.tensor.matmul(
                            ps[:, :], r32(x2t_sb[:, rb * 128:(rb + 1) * 128]),
                            r32(xt_sb[:, cc * CC:(cc + 1) * CC]),
                            start=True, stop=False)
                        nc.tensor.matmul(
                            ps[:, :], o1h_sb[:, :], a1row[:, :],
                            start=False, stop=True)
                        sch = sp1.tile([128, CC], F32, tag="sch")
                        nc.scalar.copy(sch[:, :], ps[:, :])
                        for sl in range(NSEL):
                            nc.vector.max(
                                cands[:, (cc * NSEL + sl) * 8:(cc * NSEL + sl + 1) * 8],
                                sch[:, sl * SEL:(sl + 1) * SEL])
                    # stage B: top-NTOP of candidates
                    maxima = sm1.tile([128, NTOP], F32, tag="maxima")
                    work = cands
                    for r in range(NR):
                        nc.vector.max(maxima[:, r * 8:(r + 1) * 8], work[:, :])
                        if r < NR - 1:
                            nwork = sp1.tile([128, NCAND], F32, tag="cands")
                            nc.vector.match_replace(
                                nwork[:, :], maxima[:, r * 8:(r + 1) * 8],
                                work[:, :], NEG)
                            work = nwork
                    # stats
                    d2t = sm1.tile([128, NTOP], F32, tag="d2t")
                    nc.vector.tensor_scalar(
                        d2t[:, :], maxima[:, :], -1.0, bias0_sb[:, rb:rb + 1],
                        ALU.mult, ALU.add)
                    d40 = sm1.tile([128, NTOP], F32, tag="d40")
                    nc.scalar.activation(d40[:, :], d2t[:, :], AF.Sqrt)
                    bsum = sm1.tile([128, 1], F32, tag="bsum")
                    nc.vector.reduce_sum(bsum[:, :], d40[:, 0:K], axis=mybir.AxisListType.X)
                    akt = sm1.tile([128, 1], F32, tag="akt")
                    nc.vector.tensor_scalar(
                        akt[:, :], d40[:, K:K + 1], float(K), (K + 1) * 1e-10,
                        ALU.mult, ALU.add)
                    den = sm1.tile([128, 1], F32, tag="den")
                    nc.vector.tensor_sub(den[:, :], akt[:, :], bsum[:, :])
                    dd2 = sm1.tile([128, 1], F32, tag="dd2")
                    nc.vector.tensor_scalar_mul(dd2[:, :], den[:, :], 2.0)
                    s1t = sm1.tile([128, 1], F32, tag="s1t")
                    nc.vector.reciprocal(s1t[:, :], dd2[:, :])
                    aep = sm1.tile([128, 1], F32, tag="aep")
                    nc.vector.tensor_scalar_add(aep[:, :], d40[:, K:K + 1], 1e-10)
                    nc.vector.tensor_mul(a1b_sb[:, rb:rb + 1], aep[:, :], s1t[:, :])
                    nc.vector.tensor_scalar_mul(ns1_sb[:, rb:rb + 1], s1t[:, :], -1.0)
                    nc.vector.tensor_mul(q_sb[:, rb:rb + 1], s1t[:, :], uloc_sb[:, rb:rb + 1])
                    nc.vector.tensor_mul(qa_sb[:, rb:rb + 1], aep[:, :], q_sb[:, rb:rb + 1])

            # ship q/qa to DRAM bounce, allgather
            qv = gb_in[0, 0:ROWS].rearrange("(rb p) -> p rb", p=128)
            qav = gb_in[0, ROWS:2 * ROWS].rearrange("(rb p) -> p rb", p=128)
            nc.sync.dma_start(qv, q_sb[:, :])
            nc.sync.dma_start(qav, qa_sb[:, :])
            nc.gpsimd.collective_compute(
                "AllGather", ALU.bypass, replica_groups=RG,
                ins=[gb_in[:, :].opt()], outs=[gb[:, :].opt()])

            # ================= PHASE 2: adjacency stripe =================
            CC2 = min(int(__import__('os').environ.get('KCC2', '2048')), N)
            NCC2 = N // CC2
            SUB = CC2 // 512
            with (
                tc.tile_pool(name="p2ps_s", bufs=2, space="PSUM") as pps,
                tc.tile_pool(name="p2ps_b", bufs=(1 if CC2 >= 2048 else 2), space="PSUM") as ppb,
                tc.tile_pool(name="p2ps_g", bufs=1, space="PSUM") as ppg,
                tc.tile_pool(name="p2sb", bufs=2) as sp2,
                tc.tile_pool(name="p2row", bufs=2) as rp2,
            ):
                for cc in range(NCC2):
                    qbc = sp2.tile([128, CC2], F32, tag="qbc")
                    qabc = sp2.tile([128, CC2], F32, tag="qabc")
                    pos = 0
                    while pos < CC2:
                        j = cc * CC2 + pos
                        dev, off = j // ROWS, j % ROWS
                        L = min(ROWS - off, CC2 - pos)
                        nc.sync.dma_start(
                            qbc[:, pos:pos + L],
                            gb[dev, off:off + L].bitcast(F32).partition_broadcast(128))
                        nc.sync.dma_start(
                            qabc[:, pos:pos + L],
                            gb[dev, ROWS + off:ROWS + off + L].bitcast(F32).partition_broadcast(128))
                        pos += L
                    a2row = rp2.tile([2, CC2], FP16, tag="a2row")
                    r12t = rp2.tile([12, CC2], BF16, tag="r12t")
                    nc.sync.dma_start(a2row[:, :], aug2[:, cc * CC2:(cc + 1) * CC2])
                    nc.sync.dma_start(r12t[:, :], rh12[:, cc * CC2:(cc + 1) * CC2])
                    gpss = [ppg.tile([2, 512], F32, tag=f"gps{s}", name=f"gps{s}_{cc}")
                            for s in range(SUB)]

                    SKEW = 2
                    dps = {}

                    def stage_a(rb):
                        dp = sp2.tile([128, CC2], F32, tag="dp", bufs=SKEW + 2,
                                      name=f"dp_{cc}_{rb}")
                        for sb in range(SUB):
                            ps = pps.tile([128, 512], F32, tag="ps2", bufs=2,
                                          name=f"ps_{cc}_{rb}_{sb}")
                            c0 = cc * CC2 + sb * 512
                            nc.tensor.matmul(
                                ps[:, :], r32(x2t_sb[:, rb * 128:(rb + 1) * 128]),
                                r32(xt_sb[:, c0:c0 + 512]),
                                start=True, stop=False)
                            nc.tensor.matmul(
                                ps[:, :], o2h_sb[:, :],
                                a2row[:, sb * 512:(sb + 1) * 512],
                                start=False, stop=True)
                            nc.scalar.activation(
                                dp[:, sb * 512:(sb + 1) * 512], ps[:, :], AF.Sqrt,
                                bias=bias2_sb[:, rb:rb + 1], scale=-1.0)
                        dps[rb] = dp

                    def stage_b(rb):
                        dp = dps.pop(rb)
                        a1 = sp2.tile([128, CC2], F32, tag="a1", name=f"a1_{cc}_{rb}")
                        nc.scalar.activation(a1[:, :], dp[:, :], AF.Relu,
                                             bias=a1b_sb[:, rb:rb + 1],
                                             scale=ns1_sb[:, rb:rb + 1])
                        mt = sp2.tile([128, CC2], F32R, tag="mt", name=f"mt_{cc}_{rb}")
                        nc.gpsimd.tensor_tensor(mt[:, :], dp[:, :], qbc[:, :],
                                                op=ALU.mult)
                        nc.vector.scalar_tensor_tensor(
                            mt[:, :], mt[:, :], 0.0, qabc[:, :], ALU.add, ALU.subtract)
                        nc.vector.scalar_tensor_tensor(
                            mt[:, :], mt[:, :], 0.0, a1[:, :], ALU.min, ALU.subtract)
                        adjt = sp2.tile([128, CC2], BF16, tag="adjt",
                                        name=f"adjt_{cc}_{rb}")
                        for sb in range(SUB):
                            bps = ppb.tile([128, 512], F32, tag="bps", bufs=1,
                                           name=f"bps_{cc}_{rb}_{sb}")
                            nc.tensor.matmul(
                                bps[:, :], lh12_sb[:, rb * 128:(rb + 1) * 128],
                                r12t[:, sb * 512:(sb + 1) * 512],
                                start=True, stop=True)
                            nc.vector.tensor_sub(
                                adjt[:, sb * 512:(sb + 1) * 512], bps[:, :],
                                mt[:, sb * 512:(sb + 1) * 512])
                            nc.tensor.matmul(
                                gpss[sb][:, :],
                                r32(hloc_sb[:, rb * 2:(rb + 1) * 2]),
                                mt[:, sb * 512:(sb + 1) * 512],
                                start=(rb == 0), stop=(rb == RB - 1))
                        nc.sync.dma_start(
                            o_adj[rb * 128:(rb + 1) * 128, cc * CC2:(cc + 1) * CC2],
                            adjt[:, :])

                    for it in range(RB + SKEW):
                        if it < RB:
                            stage_a(it)
                        if it >= SKEW:
                            stage_b(it - SKEW)

                    bght = rp2.tile([2, CC2], F32, tag="bght", bufs=1)
                    nc.sync.dma_start(bght[:, :], bgh[:, cc * CC2:(cc + 1) * CC2])
                    gstage = rp2.tile([2, CC2], F32, tag="gstage", bufs=1)
                    for sb in range(SUB):
                        nc.vector.tensor_sub(gstage[:, sb * 512:(sb + 1) * 512],
                                             bght[:, sb * 512:(sb + 1) * 512],
                                             gpss[sb][:, :])
                    nc.sync.dma_start(gpart0[0, cc * CC2:(cc + 1) * CC2], gstage[0:1, :])
                    nc.sync.dma_start(gpart1[0, cc * CC2:(cc + 1) * CC2], gstage[1:2, :])

            # reduce-scatter partial g to local rows (per component)
            nc.gpsimd.collective_compute(
                "ReduceScatter", ALU.add, replica_groups=RG,
                ins=[gpart0[:, :].opt()], outs=[grs0[:, :].opt()])
            nc.gpsimd.collective_compute(
                "ReduceScatter", ALU.add, replica_groups=RG,
                ins=[gpart1[:, :].opt()], outs=[grs1[:, :].opt()])

            # ================= EPILOGUE =================
            with (
                tc.tile_pool(name="e_ps", bufs=1, space="PSUM") as pe,
                tc.tile_pool(name="e_sb", bufs=1) as se,
            ):
                gpc = se.tile([128, RB * 2], F32)
                nc.sync.dma_start(
                    gpc[:, :].rearrange("p (rb c) -> c p rb", c=2)[0],
                    grs0[0, :].rearrange("(rb p) -> p rb", p=128))
                nc.sync.dma_start(
                    gpc[:, :].rearrange("p (rb c) -> c p rb", c=2)[1],
                    grs1[0, :].rearrange("(rb p) -> p rb", p=128))
                gpct = se.tile([128, RB * 2], F32R)
                nc.scalar.activation(gpct[:, :], gpc[:, :], AF.Tanh)
                nc.sync.dma_start(
                    o_g[:, :].rearrange("(rb p) c -> p rb c", p=128),
                    gpct[:, :].bitcast(F32))
                # centroid partial agg = ohm_loc.T @ tanh(g_loc)
                aggps = pe.tile([NCLS, 2], F32)
                for rb in range(RB):
                    nc.tensor.matmul(
                        aggps[:, :], r32(ohml_sb[:, rb * NCLS:(rb + 1) * NCLS]),
                        r32(gpct[:, rb * 2:(rb + 1) * 2]),
                        start=(rb == 0), stop=(rb == RB - 1))
                aggsb = se.tile([NCLS, 2], F32)
                nc.vector.tensor_copy(aggsb[:, :], aggps[:, :])
                nc.sync.dma_start(aggd[:, :], aggsb[:, :])
                nc.gpsimd.collective_compute(
                    "AllReduce", ALU.add, replica_groups=RG,
                    ins=[aggd[:, :].opt()], outs=[aggf[:, :].opt()])
                aggt = se.tile([2, NCLS], F32)
                nc.sync.dma_start(aggt[:, :], aggf[:, :].rearrange("m c -> c m"))
                ctrt = se.tile([2, NCLS], F32R)
                nc.vector.tensor_mul(ctrt[:, :], aggt[:, :], invc2_sb[:, :])
                sq2 = se.tile([2, NCLS], F32R)
                nc.vector.tensor_mul(sq2[:, :], ctrt[:, :], ctrt[:, :])
                ones2f = se.tile([2, 128], F32)
                ones2 = se.tile([2, 128], F32R)
                nc.vector.memset(ones2f[:, :], 1.0)
                nc.vector.tensor_copy(ones2[:, :], ones2f[:, :])
                crow2 = se.tile([1, 2 * NCLS], F32R)
                nc.sync.dma_start(crow2[0:1, 0:NCLS], ctrt[0:1, :])
                nc.sync.dma_start(crow2[0:1, NCLS:2 * NCLS], ctrt[1:2, :])
                bc = pe.tile([128, 3 * NCLS], F32)
                nc.tensor.matmul(bc[:, 0:NCLS], r32(ones_sb[:, :]), r32(crow2[0:1, 0:NCLS]),
                                 start=True, stop=True, skip_group_check=True)
                nc.tensor.matmul(bc[:, NCLS:2 * NCLS], r32(ones_sb[:, :]),
                                 r32(crow2[0:1, NCLS:2 * NCLS]),
                                 start=True, stop=True, skip_group_check=True)
                nc.tensor.matmul(bc[:, 2 * NCLS:3 * NCLS], r32(ones2[:, :]), r32(sq2[:, :]),
                                 start=True, stop=True, skip_group_check=True)
                c0b, c1b, csb = (bc[:, 0:NCLS], bc[:, NCLS:2 * NCLS],
                                 bc[:, 2 * NCLS:3 * NCLS])
                scall = se.tile([128, RB * NCLS], F32)
                for rb in range(RB):
                    g0 = gpct[:, rb * 2:rb * 2 + 1].bitcast(F32)
                    g1 = gpct[:, rb * 2 + 1:rb * 2 + 2].bitcast(F32)
                    sqg = se.tile([128, 2], F32, tag="sqg")
                    nc.vector.tensor_mul(sqg[:, :], gpct[:, rb * 2:(rb + 1) * 2],
                                         gpct[:, rb * 2:(rb + 1) * 2])
                    gsq = se.tile([128, 1], F32, tag="gsq")
                    nc.vector.reduce_sum(gsq[:, :], sqg[:, :], axis=mybir.AxisListType.X)
                    v1 = se.tile([128, NCLS], F32, tag="v1")
                    nc.vector.tensor_scalar(v1[:, :], c1b, g1, None, ALU.mult)
                    v2 = se.tile([128, NCLS], F32, tag="v2")
                    nc.vector.scalar_tensor_tensor(
                        v2[:, :], c0b, g0, v1[:, :], ALU.mult, ALU.add)
                    v3 = se.tile([128, NCLS], F32, tag="v3")
                    nc.vector.scalar_tensor_tensor(
                        v3[:, :], v2[:, :], -2.0, csb, ALU.mult, ALU.add)
                    v4 = se.tile([128, NCLS], F32, tag="v4")
                    nc.vector.tensor_scalar(
                        v4[:, :], v3[:, :], gsq[:, :], 0.0, ALU.add, ALU.max)
                    v5 = se.tile([128, NCLS], F32, tag="v5")
                    nc.scalar.activation(v5[:, :], v4[:, :], AF.Sqrt)
                    nc.vector.tensor_scalar_mul(
                        scall[:, rb * NCLS:(rb + 1) * NCLS], v5[:, :], -1.0)
                nc.sync.dma_start(
                    o_sc[:, :].rearrange("(rb p) c -> p rb c", p=128), scall[:, :])
    return nc


_CACHE = {}


def _get_nc(N, CORES, H, NCLS, K, CC, SEL):
    key = (N, CORES, H, NCLS, K, CC, SEL)
    if key not in _CACHE:
        _CACHE[key] = build(N, CORES, H, NCLS, K, CC, SEL)
    return _CACHE[key]


def round_fp32r(a):
    b = np.ascontiguousarray(a, np.float32).view(np.uint32)
    lsb = (b >> 12) & 1
    r = (b + 0x7FF + lsb) & 0xFFFFF000
    return r.view(np.float32)


def prep_inputs(x, ohm_labels, W, b, spars, CORES=8):
    N, H = x.shape
    NCLS = ohm_labels.shape[1]
    ROWS = N // CORES
    RB = ROWS // 128
    x = np.asarray(x, np.float32)
    ohm = np.asarray(ohm_labels, np.float32)
    W = np.asarray(W, np.float32)
    b = np.asarray(b, np.float32)
    u = (ohm.sum(1) == 0).astype(np.float32)
    xsq = (x * x).sum(1).astype(np.float32)
    h = (x @ W.T + b).astype(np.float32)
    fc = np.float32(1.0 / max(float(u.sum()), 1.0))
    xt = np.ascontiguousarray(x.T)
    aug1 = (-xsq)[None, :].astype(np.float16)
    aug2 = np.concatenate([aug1, (-CBIG * (1.0 - u))[None, :].astype(np.float16)], 0)
    rh12 = np.concatenate(
        [ohm.T, (fc * (1.0 - u))[None, :], (fc * u)[None, :]], 0).astype(np.float32)
    counts = ohm.sum(0)
    inv = np.where(counts > 0, 1.0 / np.maximum(counts, 1.0), 0.0).astype(np.float32)
    invc2 = np.ascontiguousarray(np.stack([inv, inv], 0))

    def perm(v):  # [ROWS,...] -> [128, RB, ...] -> [128, RB*...]
        r = v.reshape(RB, 128, -1).transpose(1, 0, 2).reshape(128, -1)
        return np.ascontiguousarray(r.astype(np.float32))

    in_maps = []
    for c in range(CORES):
        rs = slice(c * ROWS, (c + 1) * ROWS)
        bias0 = xsq[rs] * (1.0 + INFL) + 1e-6 + GUARD
        hl = h[rs]
        aggl = ohm[rs].T @ hl                     # [NCLS, 2]
        t1 = (1.0 - u[rs]) @ hl                   # [2]
        t2 = u[rs] @ hl                           # [2]
        bghv = (ohm @ aggl + fc * (np.outer(u, t1) + np.outer(1.0 - u, t2)))
        in_maps.append({
            "bgh": np.ascontiguousarray(bghv.T.astype(np.float32)),
            "xt": round_fp32r(xt), "aug1": aug1,
            "aug2": aug2, "rh12": rh12.astype(ml_dtypes.bfloat16),
            "o1h": np.ones((1, 128), np.float16),
            "o2h": np.ones((2, 128), np.float16),
            "invc2": invc2,
            "x2t": round_fp32r(2.0 * x[rs].T),
            "lh12": np.concatenate(
                [ohm[rs].T, u[rs][None, :], (1.0 - u[rs])[None, :]],
                0).astype(ml_dtypes.bfloat16),
            "bias0": perm(bias0),
            "bias2": perm(bias0 + BIG * (1.0 - u[rs])),
            "uloc": perm(u[rs]),
            "hloc": round_fp32r(perm(h[rs])),
            "ohml": round_fp32r(perm(ohm[rs])),
        })
    return in_maps


def kernel(x, ohm_labels, W, b, spars):
    N, H = np.asarray(x).shape
    NCLS = np.asarray(ohm_labels).shape[1]
    CORES = 8
    K = int(spars)
    nc = _get_nc(N, CORES, H, NCLS, K, 512, 512)
    in_maps = prep_inputs(x, ohm_labels, W, b, spars, CORES)
    res = bass_utils.run_bass_kernel_spmd(nc, in_maps, core_ids=list(range(CORES)))
    outs = res.results
    adj = np.concatenate([o["o_adj"] for o in outs], 0).astype(np.float32)
    g = np.concatenate([o["o_g"] for o in outs], 0)
    sc = np.concatenate([o["o_sc"] for o in outs], 0)
    return sc, g, adj
